# revision 1
# baseline (speedup 1.0000x reference)
"""Trainium2 Bass kernel for nn_ExaoneAttention (dense transformer attention).

Full-input contract: kernel(**inputs) takes the unsharded inputs and returns
the full [B, S, D] output. Internally shards across 8 NeuronCores:
2-way data parallel over batch x 4-way tensor parallel over kv heads
(2 kv heads = 8 query heads per core). Each core computes a partial
output through its Wo row-slice; the host sums the 4 partials per batch.

All matmuls run in float32r (full PE rate, ~1e-4 relative rounding).
Attention is computed in the "scoresT" orientation (keys on partitions,
queries on the free dim) so probs feed the PV matmul with no transposes;
softmax normalization uses a ones-vector matmul partition-reduce plus a
rank-1 broadcast matmul.
"""

import contextlib
import ctypes
import os
import sys
import types

import numpy as np

# ---------------------------------------------------------------------------
# Problem constants (hardcoded per contract)
# ---------------------------------------------------------------------------
B, S, D = 2, 2048, 4096
H, HKV, HD = 32, 8, 128
G = H // HKV
THETA = 10000.0

NCORES = 8
BAT_SHARDS = 2
KV_SHARDS = 4
KVH = HKV // KV_SHARDS  # kv heads per core = 2
QH = KVH * G  # q heads per core = 8
FQ = QH * HD  # 1024
FKV = KVH * HD  # 256
DMC = D // 128  # 32 model-dim chunks

QT = 512  # query tile
NQT = S // QT  # 4
SC = 128  # sequence chunk
NSC = S // SC  # 16
DT = 512  # output d tile
NDT = D // DT  # 8

_SCALE = float(HD) ** -0.5


# ---------------------------------------------------------------------------
# Wait-count legalization: this walrus build rejects instructions carrying
# more than a small number of sync waits (fused fp32/fp32r matmul: >1;
# drain: >4). Hoist excess waits onto standalone NoOps on the same engine
# immediately before the offending instruction; AND-semantics are preserved
# by sequential same-engine execution.
# ---------------------------------------------------------------------------
def _legalize_waits(nc):
    import bass_rust
    import concourse.mybir as mybir

    counter = 0
    for f in nc.m.functions:
        for bb in f.blocks:
            il = bb.instructions
            i = 0
            while i < len(il):
                ins = il[i]
                si = ins.sync_info
                if si is None or len(si.on_wait) <= 1:
                    i += 1
                    continue
                waits = list(si.on_wait)
                pos = i
                for w in waits[1:]:
                    counter += 1
                    nop = mybir.InstNoOp(name=f"lgw-{counter}", ins=[], outs=[])
                    nop.engine = ins.engine
                    nop.sync_info = bass_rust.SyncInfo(on_wait=[w], on_update=[])
                    il.insert(pos, nop)
                    pos += 1
                    i += 1
                ins.sync_info = bass_rust.SyncInfo(
                    on_wait=waits[:1], on_update=list(si.on_update)
                )
                i += 1
    return counter


# ---------------------------------------------------------------------------
# Bass kernel builder (per-core program; same program on all 8 cores)
# ---------------------------------------------------------------------------
def _build_nc():
    import concourse.bass as bass
    import concourse.mybir as mybir
    from concourse.masks import make_identity
    from concourse.tile import TileContext

    f32 = mybir.dt.float32
    f32r = mybir.dt.float32r
    AF = mybir.ActivationFunctionType

    nc = bass.Bass()

    hiT = nc.declare_dram_parameter("hiT", [D, S], f32, isOutput=False)
    wq = nc.declare_dram_parameter("wq", [D, FQ], f32, isOutput=False)
    wk = nc.declare_dram_parameter("wk", [D, FKV], f32, isOutput=False)
    wv = nc.declare_dram_parameter("wv", [D, FKV], f32, isOutput=False)
    wo = nc.declare_dram_parameter("wo", [FQ, D], f32, isOutput=False)
    ccT = nc.declare_dram_parameter("ccT", [HD, S], f32, isOutput=False)
    ssT = nc.declare_dram_parameter("ssT", [HD, S], f32, isOutput=False)
    dmask = nc.declare_dram_parameter("dmask", [SC, G * QT], f32, isOutput=False)
    out = nc.declare_dram_parameter("out", [S, D], f32, isOutput=True)

    # internal DRAM staging
    qT_d = nc.dram_tensor("qT_d", [QH, HD, S], f32)
    kT_d = nc.dram_tensor("kT_d", [KVH, HD, S], f32)
    v_d = nc.dram_tensor("v_d", [KVH, S, HD], f32)
    ctxT_d = nc.dram_tensor("ctxT_d", [QH, HD, S], f32)

    hiT_r = hiT[:, :].bitcast(f32r).rearrange("(c p) s -> p c s", p=128)
    wq_r = wq[:, :].bitcast(f32r).rearrange("(c p) f -> p c f", p=128)
    wk_r = wk[:, :].bitcast(f32r).rearrange("(c p) f -> p c f", p=128)
    wv_r = wv[:, :].bitcast(f32r).rearrange("(c p) f -> p c f", p=128)
    wo_r = wo[:, :].bitcast(f32r).rearrange("(h p) d -> p h d", p=128)

    def rope(vec, out_sb, psum, cc, ss):
        """out = rope(psum) elementwise with cc/ss [128, W] tables."""
        t1 = rope_tmp_pool.tile(list(out_sb.shape), f32, name="rope_t1")
        vec.tensor_mul(t1, psum, cc)
        t2 = rope_tmp_pool.tile(list(out_sb.shape), f32, name="rope_t2")
        vec.tensor_mul(t2[:64], psum[64:], ss[:64])
        vec.tensor_mul(t2[64:], psum[:64], ss[64:])
        vec.tensor_sub(out_sb[:64], t1[:64], t2[:64])
        vec.tensor_add(out_sb[64:], t1[64:], t2[64:])

    with TileContext(nc) as tc, contextlib.ExitStack() as top:
        singles = top.enter_context(tc.tile_pool(name="singles", bufs=1))
        rope_tmp_pool = top.enter_context(tc.tile_pool(name="ropetmp", bufs=2))

        cc_sb = singles.tile([HD, S], f32)
        nc.sync.dma_start(out=cc_sb, in_=ccT[:, :])
        ss_sb = singles.tile([HD, S], f32)
        nc.sync.dma_start(out=ss_sb, in_=ssT[:, :])
        dm_sb = singles.tile([SC, G * QT], f32)
        nc.sync.dma_start(out=dm_sb, in_=dmask[:, :])
        ident = singles.tile([128, 128], f32)
        make_identity(nc, ident)
        ones_tmp = singles.tile([128, 128], f32)
        nc.vector.memset(ones_tmp, 1.0)
        ones_p = singles.tile([128, 1], f32r)
        nc.vector.tensor_copy(ones_p, ones_tmp[:, :1])
        ones_f = singles.tile([1, 128], f32r)
        nc.vector.tensor_copy(ones_f, ones_tmp[:1, :])

        # ---------------- Phase A: K/V projection (+rope K, transpose V) ----
        with contextlib.ExitStack() as ph:
            wkv_pool = ph.enter_context(tc.tile_pool(name="wkv", bufs=1))
            hi_pool = ph.enter_context(tc.tile_pool(name="hiA", bufs=2))
            stage_pool = ph.enter_context(tc.tile_pool(name="stageA", bufs=3))
            psA = ph.enter_context(tc.tile_pool(name="psA", bufs=4, space="PSUM"))
            psT = ph.enter_context(tc.tile_pool(name="psTr", bufs=2, space="PSUM"))

            wk_sb = wkv_pool.tile([128, DMC, FKV], f32r)
            nc.sync.dma_start(out=wk_sb, in_=wk_r)
            wv_sb = wkv_pool.tile([128, DMC, FKV], f32r)
            nc.sync.dma_start(out=wv_sb, in_=wv_r)

            for st in range(NQT):
                ssl = slice(st * QT, (st + 1) * QT)
                # two half-slabs of hiT for this s-tile (SBUF headroom)
                slabs = []
                for hh in range(2):
                    slab = hi_pool.tile([128, DMC // 2, QT], f32r, name="hiA_slab")
                    nc.sync.dma_start(
                        out=slab, in_=hiT_r[:, hh * (DMC // 2) : (hh + 1) * (DMC // 2), ssl]
                    )
                    slabs.append(slab)

                for fc in range(2 * KVH):  # k0,k1,v0,v1
                    is_k = fc < KVH
                    w_sb = wk_sb if is_k else wv_sb
                    fs = slice((fc % KVH) * 128, (fc % KVH) * 128 + 128)
                    pk = psA.tile([128, QT], f32, name="psA")
                    for c in range(DMC):
                        nc.tensor.matmul(
                            pk,
                            w_sb[:, c, fs],
                            slabs[c // (DMC // 2)][:, c % (DMC // 2), :],
                            start=(c == 0),
                            stop=(c == DMC - 1),
                        )
                    kv = fc % KVH
                    if is_k:
                        kt_sb = stage_pool.tile([128, QT], f32r, name="kt_st")
                        rope(nc.vector, kt_sb, pk, cc_sb[:, ssl], ss_sb[:, ssl])
                        nc.sync.dma_start(out=kT_d[kv, :, ssl].bitcast(f32r), in_=kt_sb)
                    else:
                        vt_st = stage_pool.tile([128, QT], f32, name="vt_st")
                        nc.scalar.copy(vt_st, pk)
                        for j in range(QT // 128):
                            ptr = psT.tile([128, 128], f32, name="ptr")
                            nc.tensor.transpose(ptr, vt_st[:, j * 128 : (j + 1) * 128], ident)
                            vblk = stage_pool.tile([128, 128], f32, name="vblk")
                            nc.scalar.copy(vblk, ptr)
                            s0 = st * QT + j * 128
                            nc.sync.dma_start(out=v_d[kv, s0 : s0 + 128, :], in_=vblk)

        # ---------------- Phase B: Q projection (+rope) ---------------------
        with contextlib.ExitStack() as ph:
            wq_pool = ph.enter_context(tc.tile_pool(name="wqp", bufs=1))
            hi_pool = ph.enter_context(tc.tile_pool(name="hiB", bufs=3))
            stage_pool = ph.enter_context(tc.tile_pool(name="stageB", bufs=3))
            psB = ph.enter_context(tc.tile_pool(name="psB", bufs=8, space="PSUM"))

            wq_sb = wq_pool.tile([128, DMC, FQ], f32r)
            nc.sync.dma_start(out=wq_sb, in_=wq_r)

            for qt in range(NQT):
                ssl = slice(qt * QT, (qt + 1) * QT)
                banks = [psB.tile([128, QT], f32, name="psB") for _ in range(QH)]
                for c in range(DMC):
                    hi_t = hi_pool.tile([128, QT], f32r, name="hiB_t")
                    nc.sync.dma_start(out=hi_t, in_=hiT_r[:, c, ssl])
                    for h in range(QH):
                        nc.tensor.matmul(
                            banks[h],
                            wq_sb[:, c, h * 128 : (h + 1) * 128],
                            hi_t,
                            start=(c == 0),
                            stop=(c == DMC - 1),
                        )
                for h in range(QH):
                    qt_sb = stage_pool.tile([128, QT], f32r, name="qt_st")
                    rope(nc.vector, qt_sb, banks[h], cc_sb[:, ssl], ss_sb[:, ssl])
                    nc.sync.dma_start(out=qT_d[h, :, ssl].bitcast(f32r), in_=qt_sb)

        # ---------------- Phase C: attention --------------------------------
        with contextlib.ExitStack() as ph:
            kv_pool = ph.enter_context(tc.tile_pool(name="kvp", bufs=1))
            q_pool = ph.enter_context(tc.tile_pool(name="qp", bufs=3))
            pt_pool = ph.enter_context(tc.tile_pool(name="ptp", bufs=3))
            acc_pool = ph.enter_context(tc.tile_pool(name="accp", bufs=2))
            misc_pool = ph.enter_context(tc.tile_pool(name="miscC", bufs=3))
            ps_s = ph.enter_context(tc.tile_pool(name="ps_s", bufs=3, space="PSUM"))
            ps_ctx = ph.enter_context(tc.tile_pool(name="ps_ctx", bufs=2, space="PSUM"))
            ps_r = ph.enter_context(tc.tile_pool(name="ps_r", bufs=1, space="PSUM"))

            kt_sb = kv_pool.tile([128, KVH, S], f32r)
            nc.sync.dma_start(out=kt_sb, in_=kT_d[:, :, :].bitcast(f32r).rearrange("k p s -> p k s"))
            v_sb = kv_pool.tile([128, KVH, NSC, HD], f32r)
            nc.sync.dma_start(
                out=v_sb,
                in_=v_d[:, :, :].bitcast(f32r).rearrange("k (sc p) d -> p k sc d", p=128),
            )

            for qt in range(NQT):
                ssl = slice(qt * QT, (qt + 1) * QT)
                nk = G * (qt + 1)
                for h in range(QH):
                    kv = h // G
                    qt_sb = q_pool.tile([128, QT], f32r, name="qt_at")
                    nc.sync.dma_start(out=qt_sb, in_=qT_d[h, :, ssl].bitcast(f32r))
                    pctx = ps_ctx.tile([128, QT], f32, name="pctx")
                    acc = acc_pool.tile([128, QT], f32r, name="acc")
                    for i in range(nk):
                        pss = ps_s.tile([128, QT], f32, name="pss")
                        nc.tensor.matmul(
                            pss,
                            kt_sb[:, kv, i * 128 : (i + 1) * 128],
                            qt_sb,
                            start=True,
                            stop=True,
                        )
                        if i >= G * qt:
                            t = i - G * qt
                            nc.vector.tensor_add(
                                pss, pss, dm_sb[:, t * QT : (t + 1) * QT]
                            )
                        pt = pt_pool.tile([128, QT], f32r, name="pt")
                        nc.scalar.activation(pt, pss, AF.Exp, scale=_SCALE)
                        nc.tensor.matmul(
                            pctx,
                            v_sb[:, kv, i, :],
                            pt,
                            start=(i == 0),
                            stop=(i == nk - 1),
                        )
                        if i == 0:
                            nc.vector.tensor_copy(acc, pt)
                        else:
                            nc.vector.tensor_add(acc, acc, pt)
                    pred = ps_r.tile([1, QT], f32, name="pred")
                    nc.tensor.matmul(pred, ones_p, acc, start=True, stop=True)
                    recip = misc_pool.tile([1, QT], f32r, name="recip")
                    with nc.allow_low_precision(reason="f32r recip: 1e-4 ok here"):
                        nc.vector.reciprocal(recip, pred)
                    pbc = ps_r.tile([128, QT], f32, name="pbc")
                    nc.tensor.matmul(pbc, ones_f, recip, start=True, stop=True)
                    bc_sb = misc_pool.tile([128, QT], f32, name="bc_sb")
                    nc.scalar.copy(bc_sb, pbc)
                    ctx_sb = misc_pool.tile([128, QT], f32r, name="ctx_sb")
                    nc.vector.tensor_mul(ctx_sb, pctx, bc_sb)
                    nc.sync.dma_start(out=ctxT_d[h, :, ssl].bitcast(f32r), in_=ctx_sb)

        # ---------------- Phase D: output projection ------------------------
        with contextlib.ExitStack() as ph:
            wo_pool = ph.enter_context(tc.tile_pool(name="wop", bufs=1))
            cx_pool = ph.enter_context(tc.tile_pool(name="cxp", bufs=3))
            o_pool = ph.enter_context(tc.tile_pool(name="op", bufs=3))
            ps_o = ph.enter_context(tc.tile_pool(name="ps_o", bufs=3, space="PSUM"))

            wo_sb = wo_pool.tile([128, QH, D], f32r)
            nc.sync.dma_start(out=wo_sb, in_=wo_r)

            for sc in range(NSC):
                cx_sb = cx_pool.tile([128, QH, 128], f32r, name="cx")
                nc.sync.dma_start(
                    out=cx_sb,
                    in_=ctxT_d[:, :, sc * 128 : (sc + 1) * 128]
                    .bitcast(f32r)
                    .rearrange("h p s -> p h s"),
                )
                for dt in range(NDT):
                    po = ps_o.tile([128, DT], f32, name="po")
                    for h in range(QH):
                        nc.tensor.matmul(
                            po,
                            cx_sb[:, h, :],
                            wo_sb[:, h, dt * DT : (dt + 1) * DT],
                            start=(h == 0),
                            stop=(h == QH - 1),
                        )
                    o_sb = o_pool.tile([128, DT], f32, name="o_sb")
                    nc.scalar.copy(o_sb, po)
                    nc.sync.dma_start(
                        out=out[sc * 128 : (sc + 1) * 128, dt * DT : (dt + 1) * DT],
                        in_=o_sb,
                    )

    _legalize_waits(nc)
    return nc


_NC_CACHE = {}
_last_exec_ns = None


def _get_nc():
    if "nc" not in _NC_CACHE:
        _NC_CACHE["nc"] = _build_nc()
    return _NC_CACHE["nc"]


# ---------------------------------------------------------------------------
# Optional NTFF profiling hook (used by the local test harness via
# KERNEL_TRACE=1; grading path leaves it off)
# ---------------------------------------------------------------------------
def _install_ntff_hook(so_path="/opt/axon/libaxon_pjrt.so"):
    if "antenv.axon_hooks" in sys.modules:
        return
    try:
        lib = ctypes.CDLL(so_path)
    except OSError:
        lib = None
    if lib is None or not hasattr(lib, "axon_start_nrt_profile"):
        hook = None
    else:
        lib.axon_start_nrt_profile.argtypes = [
            ctypes.POINTER(ctypes.c_int64),
            ctypes.c_size_t,
        ]
        lib.axon_start_nrt_profile.restype = ctypes.c_int64
        lib.axon_stop_nrt_profile.argtypes = [ctypes.c_char_p]
        lib.axon_stop_nrt_profile.restype = ctypes.c_int64

        @contextlib.contextmanager
        def hook(output_dir, device_ids):
            import jax

            jax.devices()
            if device_ids:
                ids = (ctypes.c_int64 * len(device_ids))(*device_ids)
                rc = lib.axon_start_nrt_profile(ids, len(device_ids))
            else:
                rc = lib.axon_start_nrt_profile(None, 0)
            if rc != 0:
                raise RuntimeError(f"axon_start_nrt_profile rc={rc}")
            try:
                yield
            finally:
                n = lib.axon_stop_nrt_profile(str(output_dir).encode())
                print(f"ntff profile: {n} file(s) -> {output_dir}", file=sys.stderr)

    mod = types.ModuleType("antenv.axon_hooks")
    mod.get_axon_ntff_profile_hook = lambda: hook
    sys.modules["antenv.axon_hooks"] = mod


# ---------------------------------------------------------------------------
# Host entry point
# ---------------------------------------------------------------------------
def kernel(hidden_states, position_ids, attention_mask, Wq, Wk, Wv, Wo):
    global _last_exec_ns
    from concourse import bass_utils

    hidden_states = np.asarray(hidden_states, dtype=np.float32)
    position_ids = np.asarray(position_ids)
    attention_mask = np.asarray(attention_mask)
    Wq = np.asarray(Wq, dtype=np.float32)
    Wk = np.asarray(Wk, dtype=np.float32)
    Wv = np.asarray(Wv, dtype=np.float32)
    Wo = np.asarray(Wo, dtype=np.float32)

    if not np.all(np.asarray(attention_mask) > 0):
        # Spec guarantees an all-ones mask; fall back to a host reference
        # implementation for the general case rather than mis-computing.
        return _host_reference(
            hidden_states, position_ids, attention_mask, Wq, Wk, Wv, Wo
        )

    # rope tables per batch: cc/ss [HD, S] with halves stacked
    half = HD // 2
    inv_freq = 1.0 / (THETA ** (np.arange(0, half, dtype=np.float32) / half))
    ccs, sss = [], []
    for b in range(B):
        freqs = position_ids[b].astype(np.float32)[:, None] * inv_freq[None, :]
        cosT = np.cos(freqs).T.astype(np.float32)  # [64, S]
        sinT = np.sin(freqs).T.astype(np.float32)
        ccs.append(np.ascontiguousarray(np.concatenate([cosT, cosT], axis=0)))
        sss.append(np.ascontiguousarray(np.concatenate([sinT, sinT], axis=0)))

    # causal diagonal masks: block t in [0, G): dmask[kk, t*QT + qq] = 0 if
    # qq >= t*128 + kk else -1e30  (pre-scale additive, exp -> 0)
    kk = np.arange(SC)[:, None]
    qq = np.arange(QT)[None, :]
    dmask = np.concatenate(
        [
            np.where(qq >= t * SC + kk, 0.0, -1.0e30).astype(np.float32)
            for t in range(G)
        ],
        axis=1,
    )
    dmask = np.ascontiguousarray(dmask)

    hiTs = [np.ascontiguousarray(hidden_states[b].T) for b in range(B)]

    in_maps = []
    for c in range(NCORES):
        b = c // KV_SHARDS
        m = c % KV_SHARDS
        qcols = slice(m * FQ, (m + 1) * FQ)
        kvcols = slice(m * FKV, (m + 1) * FKV)
        in_maps.append(
            {
                "hiT": hiTs[b],
                "wq": np.ascontiguousarray(Wq[:, qcols]),
                "wk": np.ascontiguousarray(Wk[:, kvcols]),
                "wv": np.ascontiguousarray(Wv[:, kvcols]),
                "wo": np.ascontiguousarray(Wo[qcols, :]),
                "ccT": ccs[b],
                "ssT": sss[b],
                "dmask": dmask,
            }
        )

    nc = _get_nc()
    trace = os.environ.get("KERNEL_TRACE", "") == "1"
    if trace:
        _install_ntff_hook()
        bass_utils.upload_artifacts = lambda tmpdir: f"local:{tmpdir}"
    res = bass_utils.run_bass_kernel_spmd(
        nc, in_maps, list(range(NCORES)), trace=trace
    )
    _last_exec_ns = res.exec_time_ns

    out = np.zeros((B, S, D), dtype=np.float32)
    for c in range(NCORES):
        out[c // KV_SHARDS] += res.results[c]["out"]
    return out


def _host_reference(hidden_states, position_ids, attention_mask, Wq, Wk, Wv, Wo):
    """Numpy fallback for inputs outside the spec's guarantees."""
    q = (hidden_states @ Wq).reshape(B, S, H, HD)
    k = (hidden_states @ Wk).reshape(B, S, HKV, HD)
    v = (hidden_states @ Wv).reshape(B, S, HKV, HD)

    half = HD // 2
    inv_freq = 1.0 / (THETA ** (np.arange(0, half, dtype=np.float32) / half))
    freqs = position_ids.astype(np.float32)[..., None] * inv_freq
    cos = np.cos(freqs)[:, :, None, :]
    sin = np.sin(freqs)[:, :, None, :]

    def rope(x):
        x1, x2 = x[..., :half], x[..., half:]
        return np.concatenate([x1 * cos - x2 * sin, x2 * cos + x1 * sin], axis=-1)

    q, k = rope(q), rope(k)
    qg = q.reshape(B, S, HKV, G, HD)
    scores = np.einsum("bqhgd,bkhd->bhgqk", qg, k) * (HD**-0.5)
    causal = np.tril(np.ones((S, S), bool))
    mask = causal[None, None, None] & (attention_mask[:, None, None, None, :] > 0)
    scores = np.where(mask, scores, np.finfo(np.float32).min)
    scores = scores - scores.max(axis=-1, keepdims=True)
    probs = np.exp(scores)
    probs = probs / probs.sum(axis=-1, keepdims=True)
    ctx = np.einsum("bhgqk,bkhd->bqhgd", probs, v).reshape(B, S, H * HD)
    return (ctx @ Wo).astype(np.float32)



# revision 9
# speedup vs baseline: 1.7191x; 1.7191x over previous
"""Trainium2 Bass kernel for nn_ExaoneAttention (dense transformer attention).

Full-input contract: kernel(**inputs) takes the unsharded inputs and returns
the full [B, S, D] output. Internally shards across 8 NeuronCores:
2-way data parallel over batch x 4-way tensor parallel over kv heads
(2 kv heads = 8 query heads per core). Each core computes a partial
output through its Wo row-slice; the host sums the 4 partials per batch.

v2 design (vs the staged f32r baseline):
- fp16 operands everywhere (PE full rate + FWL weight-load hiding, which
  f32r disables; quantization noise ~2^-11 stays well inside the 2e-2 gate).
- Single fused pipeline per 512-query s-tile: QKV projection -> rope ->
  causal attention -> output projection, all SBUF-resident (no DRAM
  staging round trips). K/V accumulate into resident SBUF tiles; the Tile
  scheduler overlaps proj(st+1) matmuls into attention(st)'s exp stalls.
- V is projected directly in [seq, head_dim] orientation (hidden chunk as
  the stationary operand) so no PE transposes are needed.
- Causal masking is a multiplicative 0/1 fp16 mask applied after exp (2x
  DVE rate); softmax denominator accumulates in fp16 (<=16 adds, then an
  exact f32 ones-matmul partition reduce); reciprocal via the fast DVE
  approximation (~18 bits), broadcast back over partitions with a rank-1
  matmul.
"""

import contextlib
import ctypes
import os
import sys
import types

import numpy as np

# ---------------------------------------------------------------------------
# Problem constants (hardcoded per contract)
# ---------------------------------------------------------------------------
B, S, D = 2, 2048, 4096
H, HKV, HD = 32, 8, 128
G = H // HKV
THETA = 10000.0

NCORES = 8
BAT_SHARDS = 2
KV_SHARDS = 4
KVH = HKV // KV_SHARDS  # kv heads per core = 2
QH = KVH * G  # q heads per core = 8
DMC = D // 128  # 32 model-dim chunks
HALF = DMC // 2  # chunks per hidden slab

QT = 512  # query tile
NQT = S // QT  # 4
SC = 128  # key chunk
NSC = S // SC  # 16
DT = 512  # output d tile
NDT = D // DT  # 8

_SCALE = float(HD) ** -0.5


# ---------------------------------------------------------------------------
# Wait-count legalization: this walrus build rejects instructions carrying
# more than a small number of sync waits (fused fp32/fp32r matmul: >1;
# drain: >4). Hoist excess waits onto standalone NoOps on the same engine
# immediately before the offending instruction; AND-semantics are preserved
# by sequential same-engine execution.
# ---------------------------------------------------------------------------
def _legalize_waits(nc):
    import bass_rust
    import concourse.mybir as mybir

    counter = 0
    for f in nc.m.functions:
        for bb in f.blocks:
            il = bb.instructions
            i = 0
            while i < len(il):
                ins = il[i]
                si = ins.sync_info
                if si is None or len(si.on_wait) <= 1:
                    i += 1
                    continue
                waits = list(si.on_wait)
                pos = i
                for w in waits[1:]:
                    counter += 1
                    nop = mybir.InstNoOp(name=f"lgw-{counter}", ins=[], outs=[])
                    nop.engine = ins.engine
                    nop.sync_info = bass_rust.SyncInfo(on_wait=[w], on_update=[])
                    il.insert(pos, nop)
                    pos += 1
                    i += 1
                ins.sync_info = bass_rust.SyncInfo(
                    on_wait=waits[:1], on_update=list(si.on_update)
                )
                i += 1
    return counter


# ---------------------------------------------------------------------------
# Bass kernel builder (per-core program; same program on all 8 cores)
# ---------------------------------------------------------------------------
def _build_nc():
    import concourse.bass as bass
    import concourse.mybir as mybir
    from concourse.tile import TileContext

    f32 = mybir.dt.float32
    f16 = mybir.dt.float16
    AF = mybir.ActivationFunctionType

    nc = bass.Bass()

    # host-prearranged layouts (partition dim first everywhere)
    hi = nc.declare_dram_parameter("hi", [128, NQT, 2, HALF, QT], f16, isOutput=False)
    wq = nc.declare_dram_parameter("wq", [128, QH, DMC, 128], f16, isOutput=False)
    wk = nc.declare_dram_parameter("wk", [128, DMC, KVH * HD], f16, isOutput=False)
    wv = nc.declare_dram_parameter("wv", [128, DMC, KVH * HD], f16, isOutput=False)
    wo = nc.declare_dram_parameter("wo", [128, NDT, QH, DT], f16, isOutput=False)
    cc = nc.declare_dram_parameter("cc", [HD, S], f32, isOutput=False)
    ssn = nc.declare_dram_parameter("ssn", [HD, S], f32, isOutput=False)
    dmask = nc.declare_dram_parameter("dmask", [SC, G * QT], f16, isOutput=False)
    out = nc.declare_dram_parameter("out", [S, D], f16, isOutput=True)

    with TileContext(nc) as tc, contextlib.ExitStack() as top:
        singles = top.enter_context(tc.tile_pool(name="singles", bufs=1))
        hi_pool = top.enter_context(tc.tile_pool(name="hip", bufs=2))
        wq_pool = top.enter_context(tc.tile_pool(name="wqp", bufs=2))
        wo_pool = top.enter_context(tc.tile_pool(name="wop", bufs=2))
        qt_pool = top.enter_context(tc.tile_pool(name="qtp", bufs=2))
        ctx_pool = top.enter_context(tc.tile_pool(name="ctxp", bufs=2))
        rope_pool = top.enter_context(tc.tile_pool(name="ropep", bufs=2))
        pt_pool = top.enter_context(tc.tile_pool(name="ptp", bufs=4))
        acc_pool = top.enter_context(tc.tile_pool(name="accp", bufs=2))
        misc_pool = top.enter_context(tc.tile_pool(name="miscp", bufs=2))
        o_pool = top.enter_context(tc.tile_pool(name="op", bufs=3))
        ps_mm = top.enter_context(tc.tile_pool(name="ps_mm", bufs=2, space="PSUM"))
        ps_po = top.enter_context(tc.tile_pool(name="ps_po", bufs=2, space="PSUM"))
        ps_s = top.enter_context(tc.tile_pool(name="ps_s", bufs=2, space="PSUM"))
        ps_ctx = top.enter_context(tc.tile_pool(name="ps_ctx", bufs=1, space="PSUM"))
        ps_pb = top.enter_context(tc.tile_pool(name="ps_pb", bufs=1, space="PSUM"))

        wk_sb = singles.tile([128, DMC, KVH * HD], f16)
        nc.sync.dma_start(out=wk_sb, in_=wk[:, :, :])
        wv_sb = singles.tile([128, DMC, KVH * HD], f16)
        nc.sync.dma_start(out=wv_sb, in_=wv[:, :, :])
        cc_sb = singles.tile([HD, S], f32)
        nc.sync.dma_start(out=cc_sb, in_=cc[:, :])
        ssn_sb = singles.tile([HD, S], f32)
        nc.sync.dma_start(out=ssn_sb, in_=ssn[:, :])
        dm_sb = singles.tile([SC, G * QT], f16)
        nc.sync.dma_start(out=dm_sb, in_=dmask[:, :])
        kT_sb = singles.tile([128, KVH, S], f16)
        v_sb = singles.tile([128, NSC, KVH, HD], f16)
        ones_p = singles.tile([128, 1], f16)
        nc.vector.memset(ones_p, 1.0)
        ones_r = singles.tile([1, 128], f32)
        nc.vector.memset(ones_r, 1.0)
        nbias = singles.tile([128, 1], f32)
        nc.vector.memset(nbias, -4.0)

        def rope(dst, psum, ssl):
            """dst(f16) = neox-rope(psum) using cc and sign-folded ssn."""
            t1 = rope_pool.tile([HD, QT], f32, name="t1")
            t2 = rope_pool.tile([HD, QT], f32, name="t2")
            nc.vector.tensor_mul(t1, psum, cc_sb[:, ssl])
            nc.vector.tensor_mul(t2[:64], psum[64:], ssn_sb[:64, ssl])
            nc.vector.tensor_mul(t2[64:], psum[:64], ssn_sb[64:, ssl])
            nc.vector.tensor_add(dst, t1, t2)

        for st in range(NQT):
            ssl = slice(st * QT, (st + 1) * QT)

            slabs = []
            for hh in range(2):
                slab = hi_pool.tile([128, HALF, QT], f16, name="slab")
                nc.sync.dma_start(out=slab, in_=hi[:, st, hh])
                slabs.append(slab)

            # ---- K projection (+rope) into resident kT_sb ----
            for kv in range(KVH):
                pk = ps_mm.tile([128, QT], f32, name="mm")
                for c in range(DMC):
                    nc.tensor.matmul(
                        pk,
                        wk_sb[:, c, kv * HD : (kv + 1) * HD],
                        slabs[c // HALF][:, c % HALF, :],
                        start=(c == 0),
                        stop=(c == DMC - 1),
                    )
                rope(kT_sb[:, kv, ssl], pk, ssl)

            # ---- V projection, direct [seq, kv*HD] orientation ----
            for blk in range(QT // SC):
                pv = ps_mm.tile([128, KVH * HD], f32, name="mm")
                for c in range(DMC):
                    nc.tensor.matmul(
                        pv,
                        slabs[c // HALF][:, c % HALF, blk * SC : (blk + 1) * SC],
                        wv_sb[:, c, :],
                        start=(c == 0),
                        stop=(c == DMC - 1),
                    )
                nc.scalar.copy(v_sb[:, st * (QT // SC) + blk, :, :], pv)

            # ---- Q projection (+rope), wq streamed per head ----
            qt_t = qt_pool.tile([128, QH, QT], f16, name="qt")
            for h in range(QH):
                wqh = wq_pool.tile([128, DMC, 128], f16, name="wqh")
                nc.sync.dma_start(out=wqh, in_=wq[:, h])
                pq = ps_mm.tile([128, QT], f32, name="mm")
                for c in range(DMC):
                    nc.tensor.matmul(
                        pq,
                        wqh[:, c, :],
                        slabs[c // HALF][:, c % HALF, :],
                        start=(c == 0),
                        stop=(c == DMC - 1),
                    )
                rope(qt_t[:, h, :], pq, ssl)

            # ---- attention for this query tile ----
            ctx_t = ctx_pool.tile([128, QH, QT], f16, name="ctx")
            nk = G * (st + 1)
            for h in range(QH):
                kv = h // G
                pctx = ps_ctx.tile([128, QT], f32, name="cx")
                acc = acc_pool.tile([SC, QT], f16, name="acc")
                for i in range(nk):
                    pss = ps_s.tile([SC, QT], f32, name="ss")
                    nc.tensor.matmul(
                        pss,
                        kT_sb[:, kv, i * SC : (i + 1) * SC],
                        qt_t[:, h, :],
                        start=True,
                        stop=True,
                    )
                    pt = pt_pool.tile([SC, QT], f16, name="pt")
                    # bias -4 keeps exp inside fp16 range for extreme score
                    # tails (overflow at s*scale > 15.1 instead of 11.1); the
                    # e^-4 factor cancels exactly in the softmax normalization.
                    nc.scalar.activation(pt, pss, AF.Exp, scale=_SCALE, bias=nbias)
                    if i >= G * st:
                        t = i - G * st
                        nc.vector.tensor_mul(pt, pt, dm_sb[:, t * QT : (t + 1) * QT])
                    if i == 0:
                        nc.vector.tensor_copy(acc, pt)
                    else:
                        nc.vector.tensor_add(acc, acc, pt)
                    nc.tensor.matmul(
                        pctx,
                        v_sb[:, i, kv, :],
                        pt,
                        start=(i == 0),
                        stop=(i == nk - 1),
                    )
                pred = ps_pb.tile([1, QT], f32, name="pb")
                nc.tensor.matmul(pred, ones_p, acc, start=True, stop=True)
                # 1/x via exp(-log(x)) on ScalarE (~2 ULP each; the DVE
                # reciprocal is ~4us per call and the fast custom-DVE variant
                # does not encode on this walrus build)
                ltmp = misc_pool.tile([1, QT], f32, name="ltmp")
                nc.scalar.activation(ltmp, pred, AF.Ln)
                recip32 = misc_pool.tile([1, QT], f32, name="recip32")
                nc.scalar.activation(recip32, ltmp, AF.Exp, scale=-1.0)
                pbc = ps_pb.tile([128, QT], f32, name="pb")
                nc.tensor.matmul(pbc, ones_r, recip32, start=True, stop=True)
                bc = misc_pool.tile([128, QT], f32, name="bc")
                nc.scalar.copy(bc, pbc)
                nc.vector.tensor_mul(ctx_t[:, h, :], pctx, bc)

            # ---- output projection for this s-tile, wo streamed per d-tile ----
            for dt in range(NDT):
                wot = wo_pool.tile([128, QH, DT], f16, name="wot")
                nc.sync.dma_start(out=wot, in_=wo[:, dt])
                for blk in range(QT // SC):
                    po = ps_po.tile([SC, DT], f32, name="po")
                    for h in range(QH):
                        nc.tensor.matmul(
                            po,
                            ctx_t[:, h, blk * SC : (blk + 1) * SC],
                            wot[:, h, :],
                            start=(h == 0),
                            stop=(h == QH - 1),
                        )
                    osb = o_pool.tile([SC, DT], f16, name="osb")
                    nc.scalar.copy(osb, po)
                    r0 = st * QT + blk * SC
                    nc.sync.dma_start(
                        out=out[r0 : r0 + SC, dt * DT : (dt + 1) * DT], in_=osb
                    )

    _legalize_waits(nc)
    return nc


_NC_CACHE = {}
_last_exec_ns = None


def _get_nc():
    if "nc" not in _NC_CACHE:
        _NC_CACHE["nc"] = _build_nc()
    return _NC_CACHE["nc"]


# ---------------------------------------------------------------------------
# Optional NTFF profiling hook (used by the local test harness via
# KERNEL_TRACE=1; grading path leaves it off)
# ---------------------------------------------------------------------------
def _install_ntff_hook(so_path="/opt/axon/libaxon_pjrt.so"):
    if "antenv.axon_hooks" in sys.modules:
        return
    try:
        lib = ctypes.CDLL(so_path)
    except OSError:
        lib = None
    if lib is None or not hasattr(lib, "axon_start_nrt_profile"):
        hook = None
    else:
        lib.axon_start_nrt_profile.argtypes = [
            ctypes.POINTER(ctypes.c_int64),
            ctypes.c_size_t,
        ]
        lib.axon_start_nrt_profile.restype = ctypes.c_int64
        lib.axon_stop_nrt_profile.argtypes = [ctypes.c_char_p]
        lib.axon_stop_nrt_profile.restype = ctypes.c_int64

        @contextlib.contextmanager
        def hook(output_dir, device_ids):
            import jax

            jax.devices()
            if device_ids:
                ids = (ctypes.c_int64 * len(device_ids))(*device_ids)
                rc = lib.axon_start_nrt_profile(ids, len(device_ids))
            else:
                rc = lib.axon_start_nrt_profile(None, 0)
            if rc != 0:
                raise RuntimeError(f"axon_start_nrt_profile rc={rc}")
            try:
                yield
            finally:
                n = lib.axon_stop_nrt_profile(str(output_dir).encode())
                print(f"ntff profile: {n} file(s) -> {output_dir}", file=sys.stderr)

    mod = types.ModuleType("antenv.axon_hooks")
    mod.get_axon_ntff_profile_hook = lambda: hook
    sys.modules["antenv.axon_hooks"] = mod


# ---------------------------------------------------------------------------
# Host entry point
# ---------------------------------------------------------------------------
def kernel(hidden_states, position_ids, attention_mask, Wq, Wk, Wv, Wo):
    global _last_exec_ns
    from concourse import bass_utils

    hidden_states = np.asarray(hidden_states, dtype=np.float32)
    position_ids = np.asarray(position_ids)
    attention_mask = np.asarray(attention_mask)
    Wq = np.asarray(Wq, dtype=np.float32)
    Wk = np.asarray(Wk, dtype=np.float32)
    Wv = np.asarray(Wv, dtype=np.float32)
    Wo = np.asarray(Wo, dtype=np.float32)

    if not np.all(np.asarray(attention_mask) > 0):
        # Spec guarantees an all-ones mask; fall back to a host reference
        # implementation for the general case rather than mis-computing.
        return _host_reference(
            hidden_states, position_ids, attention_mask, Wq, Wk, Wv, Wo
        )

    # rope tables per batch: cc = [cos; cos], ssn = [-sin; sin]  (f32 [HD, S])
    half = HD // 2
    inv_freq = 1.0 / (THETA ** (np.arange(0, half, dtype=np.float32) / half))
    ccs, ssns = [], []
    for b in range(B):
        freqs = position_ids[b].astype(np.float32)[:, None] * inv_freq[None, :]
        cosT = np.cos(freqs).T.astype(np.float32)  # [64, S]
        sinT = np.sin(freqs).T.astype(np.float32)
        ccs.append(np.ascontiguousarray(np.concatenate([cosT, cosT], axis=0)))
        ssns.append(np.ascontiguousarray(np.concatenate([-sinT, sinT], axis=0)))

    # multiplicative causal masks for the diagonal blocks: block t in [0, G):
    # dmask[kk, t*QT + qq] = 1 if qq >= t*SC + kk else 0
    kk = np.arange(SC)[:, None]
    qq = np.arange(QT)[None, :]
    dmask = np.concatenate(
        [
            np.where(qq >= t * SC + kk, 1.0, 0.0).astype(np.float16)
            for t in range(G)
        ],
        axis=1,
    )
    dmask = np.ascontiguousarray(dmask)

    # hidden: [p, st, half, c_local, x]  (d = c*128 + p, s = st*QT + x)
    his = []
    for b in range(B):
        hiT = hidden_states[b].T.astype(np.float16)  # [D, S]
        t = hiT.reshape(DMC, 128, NQT, QT).transpose(1, 2, 0, 3)  # [p, st, c, x]
        his.append(np.ascontiguousarray(t.reshape(128, NQT, 2, HALF, QT)))

    in_maps = []
    for c in range(NCORES):
        b = c // KV_SHARDS
        m = c % KV_SHARDS
        FQ = QH * HD
        FKV = KVH * HD
        wq_s = Wq[:, m * FQ : (m + 1) * FQ].astype(np.float16)
        wq_pre = np.ascontiguousarray(
            wq_s.reshape(DMC, 128, QH, HD).transpose(1, 2, 0, 3)
        )  # [p, h, c, x]
        wk_s = Wk[:, m * FKV : (m + 1) * FKV].astype(np.float16)
        wk_pre = np.ascontiguousarray(
            wk_s.reshape(DMC, 128, FKV).transpose(1, 0, 2)
        )  # [p, c, kv*HD]
        wv_s = Wv[:, m * FKV : (m + 1) * FKV].astype(np.float16)
        wv_pre = np.ascontiguousarray(
            wv_s.reshape(DMC, 128, FKV).transpose(1, 0, 2)
        )
        wo_s = Wo[m * FQ : (m + 1) * FQ, :].astype(np.float16)
        wo_pre = np.ascontiguousarray(
            wo_s.reshape(QH, 128, NDT, DT).transpose(1, 2, 0, 3)
        )  # [p, dt, h, x]
        in_maps.append(
            {
                "hi": his[b],
                "wq": wq_pre,
                "wk": wk_pre,
                "wv": wv_pre,
                "wo": wo_pre,
                "cc": ccs[b],
                "ssn": ssns[b],
                "dmask": dmask,
            }
        )

    nc = _get_nc()
    trace = os.environ.get("KERNEL_TRACE", "") == "1"
    if trace:
        _install_ntff_hook()
        bass_utils.upload_artifacts = lambda tmpdir: f"local:{tmpdir}"
    res = bass_utils.run_bass_kernel_spmd(
        nc, in_maps, list(range(NCORES)), trace=trace
    )
    _last_exec_ns = res.exec_time_ns

    out = np.zeros((B, S, D), dtype=np.float32)
    for c in range(NCORES):
        out[c // KV_SHARDS] += np.asarray(res.results[c]["out"], dtype=np.float32)
    return out


def _host_reference(hidden_states, position_ids, attention_mask, Wq, Wk, Wv, Wo):
    """Numpy fallback for inputs outside the spec's guarantees."""
    q = (hidden_states @ Wq).reshape(B, S, H, HD)
    k = (hidden_states @ Wk).reshape(B, S, HKV, HD)
    v = (hidden_states @ Wv).reshape(B, S, HKV, HD)

    half = HD // 2
    inv_freq = 1.0 / (THETA ** (np.arange(0, half, dtype=np.float32) / half))
    freqs = position_ids.astype(np.float32)[..., None] * inv_freq
    cos = np.cos(freqs)[:, :, None, :]
    sin = np.sin(freqs)[:, :, None, :]

    def rope(x):
        x1, x2 = x[..., :half], x[..., half:]
        return np.concatenate([x1 * cos - x2 * sin, x2 * cos + x1 * sin], axis=-1)

    q, k = rope(q), rope(k)
    qg = q.reshape(B, S, HKV, G, HD)
    scores = np.einsum("bqhgd,bkhd->bhgqk", qg, k) * (HD**-0.5)
    causal = np.tril(np.ones((S, S), bool))
    mask = causal[None, None, None] & (attention_mask[:, None, None, None, :] > 0)
    scores = np.where(mask, scores, np.finfo(np.float32).min)
    scores = scores - scores.max(axis=-1, keepdims=True)
    probs = np.exp(scores)
    probs = probs / probs.sum(axis=-1, keepdims=True)
    ctx = np.einsum("bhgqk,bkhd->bqhgd", probs, v).reshape(B, S, H * HD)
    return (ctx @ Wo).astype(np.float32)


# revision 16
# speedup vs baseline: 1.7572x; 1.0221x over previous
"""Trainium2 Bass kernel for nn_ExaoneAttention (dense transformer attention).

Full-input contract: kernel(**inputs) takes the unsharded inputs and returns
the full [B, S, D] output. Internally shards across 8 NeuronCores:
2-way data parallel over batch x 4-way tensor parallel over kv heads
(2 kv heads = 8 query heads per core). Each core computes a partial
output through its Wo row-slice; the host sums the 4 partials per batch.

v2 design (vs the staged f32r baseline):
- fp16 operands everywhere (PE full rate + FWL weight-load hiding, which
  f32r disables; quantization noise ~2^-11 stays well inside the 2e-2 gate).
- Single fused pipeline per 512-query s-tile: QKV projection -> rope ->
  causal attention -> output projection, all SBUF-resident (no DRAM
  staging round trips). K/V accumulate into resident SBUF tiles; the Tile
  scheduler overlaps proj(st+1) matmuls into attention(st)'s exp stalls.
- V is projected directly in [seq, head_dim] orientation (hidden chunk as
  the stationary operand) so no PE transposes are needed.
- Causal masking is a multiplicative 0/1 fp16 mask applied after exp (2x
  DVE rate); softmax denominator accumulates in fp16 (<=16 adds, then an
  exact f32 ones-matmul partition reduce); reciprocal via the fast DVE
  approximation (~18 bits), broadcast back over partitions with a rank-1
  matmul.
"""

import contextlib
import ctypes
import os
import sys
import types

import numpy as np

# ---------------------------------------------------------------------------
# Problem constants (hardcoded per contract)
# ---------------------------------------------------------------------------
B, S, D = 2, 2048, 4096
H, HKV, HD = 32, 8, 128
G = H // HKV
THETA = 10000.0

NCORES = 8
BAT_SHARDS = 2
KV_SHARDS = 4
KVH = HKV // KV_SHARDS  # kv heads per core = 2
QH = KVH * G  # q heads per core = 8
DMC = D // 128  # 32 model-dim chunks
HALF = DMC // 2  # chunks per hidden slab

QT = 512  # query tile
NQT = S // QT  # 4
SC = 128  # key chunk
NSC = S // SC  # 16
DT = 512  # output d tile
NDT = D // DT  # 8

_SCALE = float(HD) ** -0.5


# ---------------------------------------------------------------------------
# Wait-count legalization: this walrus build rejects instructions carrying
# more than a small number of sync waits (fused fp32/fp32r matmul: >1;
# drain: >4). Hoist excess waits onto standalone NoOps on the same engine
# immediately before the offending instruction; AND-semantics are preserved
# by sequential same-engine execution.
# ---------------------------------------------------------------------------
def _legalize_waits(nc):
    import bass_rust
    import concourse.mybir as mybir

    counter = 0
    for f in nc.m.functions:
        for bb in f.blocks:
            il = bb.instructions
            i = 0
            while i < len(il):
                ins = il[i]
                si = ins.sync_info
                if si is None or len(si.on_wait) <= 1:
                    i += 1
                    continue
                waits = list(si.on_wait)
                pos = i
                for w in waits[1:]:
                    counter += 1
                    nop = mybir.InstNoOp(name=f"lgw-{counter}", ins=[], outs=[])
                    nop.engine = ins.engine
                    nop.sync_info = bass_rust.SyncInfo(on_wait=[w], on_update=[])
                    il.insert(pos, nop)
                    pos += 1
                    i += 1
                ins.sync_info = bass_rust.SyncInfo(
                    on_wait=waits[:1], on_update=list(si.on_update)
                )
                i += 1
    return counter


# ---------------------------------------------------------------------------
# Bass kernel builder (per-core program; same program on all 8 cores)
# ---------------------------------------------------------------------------
def _build_nc():
    import concourse.bass as bass
    import concourse.mybir as mybir
    from concourse.tile import TileContext

    f32 = mybir.dt.float32
    f16 = mybir.dt.float16
    AF = mybir.ActivationFunctionType

    nc = bass.Bass()

    # host-prearranged layouts (partition dim first everywhere)
    hi = nc.declare_dram_parameter("hi", [128, NQT, 2, HALF, QT], f16, isOutput=False)
    wq = nc.declare_dram_parameter("wq", [128, QH, DMC, 128], f16, isOutput=False)
    wk = nc.declare_dram_parameter("wk", [128, DMC, KVH * HD], f16, isOutput=False)
    wv = nc.declare_dram_parameter("wv", [128, DMC, KVH * HD], f16, isOutput=False)
    wo = nc.declare_dram_parameter("wo", [128, NDT, QH, DT], f16, isOutput=False)
    cc = nc.declare_dram_parameter("cc", [HD, S], f32, isOutput=False)
    ssn = nc.declare_dram_parameter("ssn", [HD, S], f32, isOutput=False)
    dmask = nc.declare_dram_parameter("dmask", [SC, G * QT], f16, isOutput=False)
    out = nc.declare_dram_parameter("out", [S, D], f16, isOutput=True)

    with TileContext(nc) as tc, contextlib.ExitStack() as top:
        singles = top.enter_context(tc.tile_pool(name="singles", bufs=1))
        hi_pool = top.enter_context(tc.tile_pool(name="hip", bufs=2))
        wq_pool = top.enter_context(tc.tile_pool(name="wqp", bufs=2))
        wo_pool = top.enter_context(tc.tile_pool(name="wop", bufs=2))
        qt_pool = top.enter_context(tc.tile_pool(name="qtp", bufs=2))
        ctx_pool = top.enter_context(tc.tile_pool(name="ctxp", bufs=2))
        rope_pool = top.enter_context(tc.tile_pool(name="ropep", bufs=2))
        pt_pool = top.enter_context(tc.tile_pool(name="ptp", bufs=4))
        acc_pool = top.enter_context(tc.tile_pool(name="accp", bufs=2))
        misc_pool = top.enter_context(tc.tile_pool(name="miscp", bufs=2))
        o_pool = top.enter_context(tc.tile_pool(name="op", bufs=3))
        ps_mm = top.enter_context(tc.tile_pool(name="ps_mm", bufs=2, space="PSUM"))
        ps_po = top.enter_context(tc.tile_pool(name="ps_po", bufs=2, space="PSUM"))
        ps_s = top.enter_context(tc.tile_pool(name="ps_s", bufs=2, space="PSUM"))
        ps_ctx = top.enter_context(tc.tile_pool(name="ps_ctx", bufs=1, space="PSUM"))
        ps_pb = top.enter_context(tc.tile_pool(name="ps_pb", bufs=1, space="PSUM"))

        wk_sb = singles.tile([128, DMC, KVH * HD], f16)
        nc.sync.dma_start(out=wk_sb, in_=wk[:, :, :])
        wv_sb = singles.tile([128, DMC, KVH * HD], f16)
        nc.sync.dma_start(out=wv_sb, in_=wv[:, :, :])
        # cc/ssn are DMA'd per s-tile slice inside the loop (keeps the first
        # projection matmuls off the critical path of these bulk loads)
        cc_sb = singles.tile([HD, S], f32)
        ssn_sb = singles.tile([HD, S], f32)
        dm_sb = singles.tile([SC, G * QT], f16)
        nc.sync.dma_start(out=dm_sb, in_=dmask[:, :])
        kT_sb = singles.tile([128, KVH, S], f16)
        v_sb = singles.tile([128, NSC, KVH, HD], f16)
        ones_p = singles.tile([128, 1], f16)
        nc.vector.memset(ones_p, 1.0)
        ones_r = singles.tile([1, 128], f32)
        nc.vector.memset(ones_r, 1.0)
        nbias = singles.tile([128, 1], f32)
        nc.vector.memset(nbias, -4.0)

        def rope(dst, psum, ssl):
            """dst(f16) = neox-rope(psum) using cc and sign-folded ssn."""
            t1 = rope_pool.tile([HD, QT], f32, name="t1")
            t2 = rope_pool.tile([HD, QT], f32, name="t2")
            nc.vector.tensor_mul(t1, psum, cc_sb[:, ssl])
            nc.vector.tensor_mul(t2[:64], psum[64:], ssn_sb[:64, ssl])
            nc.vector.tensor_mul(t2[64:], psum[:64], ssn_sb[64:, ssl])
            nc.vector.tensor_add(dst, t1, t2)

        for st in range(NQT):
            ssl = slice(st * QT, (st + 1) * QT)

            slabs = []
            for hh in range(2):
                slab = hi_pool.tile([128, HALF, QT], f16, name="slab")
                nc.sync.dma_start(out=slab, in_=hi[:, st, hh])
                slabs.append(slab)
            nc.sync.dma_start(out=cc_sb[:, ssl], in_=cc[:, ssl])
            nc.sync.dma_start(out=ssn_sb[:, ssl], in_=ssn[:, ssl])

            # ---- K projection (+rope) into resident kT_sb ----
            for kv in range(KVH):
                pk = ps_mm.tile([128, QT], f32, name="mm")
                for c in range(DMC):
                    nc.tensor.matmul(
                        pk,
                        wk_sb[:, c, kv * HD : (kv + 1) * HD],
                        slabs[c // HALF][:, c % HALF, :],
                        start=(c == 0),
                        stop=(c == DMC - 1),
                    )
                rope(kT_sb[:, kv, ssl], pk, ssl)

            # ---- V projection, direct [seq, kv*HD] orientation ----
            for blk in range(QT // SC):
                pv = ps_mm.tile([128, KVH * HD], f32, name="mm")
                for c in range(DMC):
                    nc.tensor.matmul(
                        pv,
                        slabs[c // HALF][:, c % HALF, blk * SC : (blk + 1) * SC],
                        wv_sb[:, c, :],
                        start=(c == 0),
                        stop=(c == DMC - 1),
                    )
                nc.scalar.copy(v_sb[:, st * (QT // SC) + blk, :, :], pv)

            # ---- Q projection (+rope), wq streamed per head ----
            qt_t = qt_pool.tile([128, QH, QT], f16, name="qt")
            for h in range(QH):
                wqh = wq_pool.tile([128, DMC, 128], f16, name="wqh")
                nc.sync.dma_start(out=wqh, in_=wq[:, h])
                pq = ps_mm.tile([128, QT], f32, name="mm")
                for c in range(DMC):
                    nc.tensor.matmul(
                        pq,
                        wqh[:, c, :],
                        slabs[c // HALF][:, c % HALF, :],
                        start=(c == 0),
                        stop=(c == DMC - 1),
                    )
                rope(qt_t[:, h, :], pq, ssl)

            # ---- attention for this query tile ----
            ctx_t = ctx_pool.tile([128, QH, QT], f16, name="ctx")
            nk = G * (st + 1)
            for h in range(QH):
                kv = h // G
                pctx = ps_ctx.tile([128, QT], f32, name="cx")
                acc = acc_pool.tile([SC, QT], f16, name="acc")
                for i in range(nk):
                    # diagonal chunk t covers keys [i*SC, i*SC+SC); queries
                    # below t*SC are fully masked there -> narrow all work to
                    # the live query range [q0, QT)
                    t = i - G * st
                    q0 = t * SC if t > 0 else 0
                    qsl = slice(q0, QT)
                    pss = ps_s.tile([SC, QT], f32, name="ss")
                    nc.tensor.matmul(
                        pss[:, qsl],
                        kT_sb[:, kv, i * SC : (i + 1) * SC],
                        qt_t[:, h, qsl],
                        start=True,
                        stop=True,
                    )
                    pt = pt_pool.tile([SC, QT], f16, name="pt")
                    # bias -4 keeps exp inside fp16 range for extreme score
                    # tails (overflow at s*scale > 15.1 instead of 11.1); the
                    # e^-4 factor cancels exactly in the softmax normalization.
                    nc.scalar.activation(
                        pt[:, qsl], pss[:, qsl], AF.Exp, scale=_SCALE, bias=nbias
                    )
                    if t >= 0:
                        nc.vector.tensor_mul(
                            pt[:, qsl], pt[:, qsl], dm_sb[:, t * QT + q0 : (t + 1) * QT]
                        )
                    if i == 0:
                        nc.vector.tensor_copy(acc, pt)
                    else:
                        nc.vector.tensor_add(acc[:, qsl], acc[:, qsl], pt[:, qsl])
                    nc.tensor.matmul(
                        pctx[:, qsl],
                        v_sb[:, i, kv, :],
                        pt[:, qsl],
                        start=(i == 0),
                        stop=(i == nk - 1),
                    )
                pred = ps_pb.tile([1, QT], f32, name="pb")
                nc.tensor.matmul(pred, ones_p, acc, start=True, stop=True)
                # 1/x via exp(-log(x)) on ScalarE (~2 ULP each; the DVE
                # reciprocal is ~4us per call and the fast custom-DVE variant
                # does not encode on this walrus build)
                ltmp = misc_pool.tile([1, QT], f32, name="ltmp")
                nc.scalar.activation(ltmp, pred, AF.Ln)
                recip32 = misc_pool.tile([1, QT], f32, name="recip32")
                nc.scalar.activation(recip32, ltmp, AF.Exp, scale=-1.0)
                pbc = ps_pb.tile([128, QT], f32, name="pb")
                nc.tensor.matmul(pbc, ones_r, recip32, start=True, stop=True)
                bc = misc_pool.tile([128, QT], f32, name="bc")
                nc.scalar.copy(bc, pbc)
                nc.vector.tensor_mul(ctx_t[:, h, :], pctx, bc)

            # ---- output projection for this s-tile, wo streamed per d-tile ----
            for dt in range(NDT):
                wot = wo_pool.tile([128, QH, DT], f16, name="wot")
                nc.sync.dma_start(out=wot, in_=wo[:, dt])
                for blk in range(QT // SC):
                    po = ps_po.tile([SC, DT], f32, name="po")
                    for h in range(QH):
                        nc.tensor.matmul(
                            po,
                            ctx_t[:, h, blk * SC : (blk + 1) * SC],
                            wot[:, h, :],
                            start=(h == 0),
                            stop=(h == QH - 1),
                        )
                    osb = o_pool.tile([SC, DT], f16, name="osb")
                    nc.scalar.copy(osb, po)
                    r0 = st * QT + blk * SC
                    nc.sync.dma_start(
                        out=out[r0 : r0 + SC, dt * DT : (dt + 1) * DT], in_=osb
                    )

    _legalize_waits(nc)
    return nc


_NC_CACHE = {}
_last_exec_ns = None


def _get_nc():
    if "nc" not in _NC_CACHE:
        _NC_CACHE["nc"] = _build_nc()
    return _NC_CACHE["nc"]


# ---------------------------------------------------------------------------
# Optional NTFF profiling hook (used by the local test harness via
# KERNEL_TRACE=1; grading path leaves it off)
# ---------------------------------------------------------------------------
def _install_ntff_hook(so_path="/opt/axon/libaxon_pjrt.so"):
    if "antenv.axon_hooks" in sys.modules:
        return
    try:
        lib = ctypes.CDLL(so_path)
    except OSError:
        lib = None
    if lib is None or not hasattr(lib, "axon_start_nrt_profile"):
        hook = None
    else:
        lib.axon_start_nrt_profile.argtypes = [
            ctypes.POINTER(ctypes.c_int64),
            ctypes.c_size_t,
        ]
        lib.axon_start_nrt_profile.restype = ctypes.c_int64
        lib.axon_stop_nrt_profile.argtypes = [ctypes.c_char_p]
        lib.axon_stop_nrt_profile.restype = ctypes.c_int64

        @contextlib.contextmanager
        def hook(output_dir, device_ids):
            import jax

            jax.devices()
            if device_ids:
                ids = (ctypes.c_int64 * len(device_ids))(*device_ids)
                rc = lib.axon_start_nrt_profile(ids, len(device_ids))
            else:
                rc = lib.axon_start_nrt_profile(None, 0)
            if rc != 0:
                raise RuntimeError(f"axon_start_nrt_profile rc={rc}")
            try:
                yield
            finally:
                n = lib.axon_stop_nrt_profile(str(output_dir).encode())
                print(f"ntff profile: {n} file(s) -> {output_dir}", file=sys.stderr)

    mod = types.ModuleType("antenv.axon_hooks")
    mod.get_axon_ntff_profile_hook = lambda: hook
    sys.modules["antenv.axon_hooks"] = mod


# ---------------------------------------------------------------------------
# Host entry point
# ---------------------------------------------------------------------------
def kernel(hidden_states, position_ids, attention_mask, Wq, Wk, Wv, Wo):
    global _last_exec_ns
    from concourse import bass_utils

    hidden_states = np.asarray(hidden_states, dtype=np.float32)
    position_ids = np.asarray(position_ids)
    attention_mask = np.asarray(attention_mask)
    Wq = np.asarray(Wq, dtype=np.float32)
    Wk = np.asarray(Wk, dtype=np.float32)
    Wv = np.asarray(Wv, dtype=np.float32)
    Wo = np.asarray(Wo, dtype=np.float32)

    if not np.all(np.asarray(attention_mask) > 0):
        # Spec guarantees an all-ones mask; fall back to a host reference
        # implementation for the general case rather than mis-computing.
        return _host_reference(
            hidden_states, position_ids, attention_mask, Wq, Wk, Wv, Wo
        )

    # rope tables per batch: cc = [cos; cos], ssn = [-sin; sin]  (f32 [HD, S])
    half = HD // 2
    inv_freq = 1.0 / (THETA ** (np.arange(0, half, dtype=np.float32) / half))
    ccs, ssns = [], []
    for b in range(B):
        freqs = position_ids[b].astype(np.float32)[:, None] * inv_freq[None, :]
        cosT = np.cos(freqs).T.astype(np.float32)  # [64, S]
        sinT = np.sin(freqs).T.astype(np.float32)
        ccs.append(np.ascontiguousarray(np.concatenate([cosT, cosT], axis=0)))
        ssns.append(np.ascontiguousarray(np.concatenate([-sinT, sinT], axis=0)))

    # multiplicative causal masks for the diagonal blocks: block t in [0, G):
    # dmask[kk, t*QT + qq] = 1 if qq >= t*SC + kk else 0
    kk = np.arange(SC)[:, None]
    qq = np.arange(QT)[None, :]
    dmask = np.concatenate(
        [
            np.where(qq >= t * SC + kk, 1.0, 0.0).astype(np.float16)
            for t in range(G)
        ],
        axis=1,
    )
    dmask = np.ascontiguousarray(dmask)

    # hidden: [p, st, half, c_local, x]  (d = c*128 + p, s = st*QT + x)
    his = []
    for b in range(B):
        hiT = hidden_states[b].T.astype(np.float16)  # [D, S]
        t = hiT.reshape(DMC, 128, NQT, QT).transpose(1, 2, 0, 3)  # [p, st, c, x]
        his.append(np.ascontiguousarray(t.reshape(128, NQT, 2, HALF, QT)))

    in_maps = []
    for c in range(NCORES):
        b = c // KV_SHARDS
        m = c % KV_SHARDS
        FQ = QH * HD
        FKV = KVH * HD
        wq_s = Wq[:, m * FQ : (m + 1) * FQ].astype(np.float16)
        wq_pre = np.ascontiguousarray(
            wq_s.reshape(DMC, 128, QH, HD).transpose(1, 2, 0, 3)
        )  # [p, h, c, x]
        wk_s = Wk[:, m * FKV : (m + 1) * FKV].astype(np.float16)
        wk_pre = np.ascontiguousarray(
            wk_s.reshape(DMC, 128, FKV).transpose(1, 0, 2)
        )  # [p, c, kv*HD]
        wv_s = Wv[:, m * FKV : (m + 1) * FKV].astype(np.float16)
        wv_pre = np.ascontiguousarray(
            wv_s.reshape(DMC, 128, FKV).transpose(1, 0, 2)
        )
        wo_s = Wo[m * FQ : (m + 1) * FQ, :].astype(np.float16)
        wo_pre = np.ascontiguousarray(
            wo_s.reshape(QH, 128, NDT, DT).transpose(1, 2, 0, 3)
        )  # [p, dt, h, x]
        in_maps.append(
            {
                "hi": his[b],
                "wq": wq_pre,
                "wk": wk_pre,
                "wv": wv_pre,
                "wo": wo_pre,
                "cc": ccs[b],
                "ssn": ssns[b],
                "dmask": dmask,
            }
        )

    nc = _get_nc()
    trace = os.environ.get("KERNEL_TRACE", "") == "1"
    if trace:
        _install_ntff_hook()
        bass_utils.upload_artifacts = lambda tmpdir: f"local:{tmpdir}"
    res = bass_utils.run_bass_kernel_spmd(
        nc, in_maps, list(range(NCORES)), trace=trace
    )
    _last_exec_ns = res.exec_time_ns

    out = np.zeros((B, S, D), dtype=np.float32)
    for c in range(NCORES):
        out[c // KV_SHARDS] += np.asarray(res.results[c]["out"], dtype=np.float32)
    return out


def _host_reference(hidden_states, position_ids, attention_mask, Wq, Wk, Wv, Wo):
    """Numpy fallback for inputs outside the spec's guarantees."""
    q = (hidden_states @ Wq).reshape(B, S, H, HD)
    k = (hidden_states @ Wk).reshape(B, S, HKV, HD)
    v = (hidden_states @ Wv).reshape(B, S, HKV, HD)

    half = HD // 2
    inv_freq = 1.0 / (THETA ** (np.arange(0, half, dtype=np.float32) / half))
    freqs = position_ids.astype(np.float32)[..., None] * inv_freq
    cos = np.cos(freqs)[:, :, None, :]
    sin = np.sin(freqs)[:, :, None, :]

    def rope(x):
        x1, x2 = x[..., :half], x[..., half:]
        return np.concatenate([x1 * cos - x2 * sin, x2 * cos + x1 * sin], axis=-1)

    q, k = rope(q), rope(k)
    qg = q.reshape(B, S, HKV, G, HD)
    scores = np.einsum("bqhgd,bkhd->bhgqk", qg, k) * (HD**-0.5)
    causal = np.tril(np.ones((S, S), bool))
    mask = causal[None, None, None] & (attention_mask[:, None, None, None, :] > 0)
    scores = np.where(mask, scores, np.finfo(np.float32).min)
    scores = scores - scores.max(axis=-1, keepdims=True)
    probs = np.exp(scores)
    probs = probs / probs.sum(axis=-1, keepdims=True)
    ctx = np.einsum("bhgqk,bkhd->bqhgd", probs, v).reshape(B, S, H * HD)
    return (ctx @ Wo).astype(np.float32)


# revision 21
# speedup vs baseline: 2.0418x; 1.1620x over previous
"""Trainium2 Bass kernel for nn_ExaoneAttention (dense transformer attention).

Full-input contract: kernel(**inputs) takes the unsharded inputs and returns
the full [B, S, D] output. Internally shards across 8 NeuronCores:
2-way data parallel over batch x 4-way tensor parallel over kv heads
(2 kv heads = 8 query heads per core). Each core computes a partial
output through its Wo row-slice; the host sums the 4 partials per batch.

v2 design (vs the staged f32r baseline):
- fp16 operands everywhere (PE full rate + FWL weight-load hiding, which
  f32r disables; quantization noise ~2^-11 stays well inside the 2e-2 gate).
- Single fused pipeline per 512-query s-tile: QKV projection -> rope ->
  causal attention -> output projection, all SBUF-resident (no DRAM
  staging round trips). K/V accumulate into resident SBUF tiles; the Tile
  scheduler overlaps proj(st+1) matmuls into attention(st)'s exp stalls.
- V is projected directly in [seq, head_dim] orientation (hidden chunk as
  the stationary operand) so no PE transposes are needed.
- Causal masking is a multiplicative 0/1 fp16 mask applied after exp (2x
  DVE rate); softmax denominator accumulates in fp16 (<=16 adds, then an
  exact f32 ones-matmul partition reduce); reciprocal via the fast DVE
  approximation (~18 bits), broadcast back over partitions with a rank-1
  matmul.
"""

import contextlib
import ctypes
import os
import sys
import types

import numpy as np

# ---------------------------------------------------------------------------
# Problem constants (hardcoded per contract)
# ---------------------------------------------------------------------------
B, S, D = 2, 2048, 4096
H, HKV, HD = 32, 8, 128
G = H // HKV
THETA = 10000.0

NCORES = 8
BAT_SHARDS = 2
KV_SHARDS = 4
KVH = HKV // KV_SHARDS  # kv heads per core = 2
QH = KVH * G  # q heads per core = 8
DMC = D // 128  # 32 model-dim chunks
HALF = DMC // 2  # chunks per hidden slab

QT = 512  # query tile
NQT = S // QT  # 4
SC = 128  # key chunk
NSC = S // SC  # 16
DT = 512  # output d tile
NDT = D // DT  # 8

_SCALE = float(HD) ** -0.5


# ---------------------------------------------------------------------------
# Wait-count legalization: this walrus build rejects instructions carrying
# more than a small number of sync waits (fused fp32/fp32r matmul: >1;
# drain: >4). Hoist excess waits onto standalone NoOps on the same engine
# immediately before the offending instruction; AND-semantics are preserved
# by sequential same-engine execution.
# ---------------------------------------------------------------------------
def _legalize_waits(nc):
    import bass_rust
    import concourse.mybir as mybir

    counter = 0
    for f in nc.m.functions:
        for bb in f.blocks:
            il = bb.instructions
            i = 0
            while i < len(il):
                ins = il[i]
                si = ins.sync_info
                if si is None or len(si.on_wait) <= 1:
                    i += 1
                    continue
                waits = list(si.on_wait)
                pos = i
                for w in waits[1:]:
                    counter += 1
                    nop = mybir.InstNoOp(name=f"lgw-{counter}", ins=[], outs=[])
                    nop.engine = ins.engine
                    nop.sync_info = bass_rust.SyncInfo(on_wait=[w], on_update=[])
                    il.insert(pos, nop)
                    pos += 1
                    i += 1
                ins.sync_info = bass_rust.SyncInfo(
                    on_wait=waits[:1], on_update=list(si.on_update)
                )
                i += 1
    return counter


# ---------------------------------------------------------------------------
# Bass kernel builder (per-core program; same program on all 8 cores)
# ---------------------------------------------------------------------------
def _build_nc():
    import concourse.bass as bass
    import concourse.mybir as mybir
    from concourse.tile import TileContext

    f32 = mybir.dt.float32
    f16 = mybir.dt.float16
    AF = mybir.ActivationFunctionType

    nc = bass.Bass()

    # host-prearranged layouts (partition dim first everywhere)
    hi = nc.declare_dram_parameter("hi", [128, NQT, 2, HALF, QT], f16, isOutput=False)
    wq = nc.declare_dram_parameter("wq", [128, QH, DMC, 128], f16, isOutput=False)
    wk = nc.declare_dram_parameter("wk", [128, DMC, KVH * HD], f16, isOutput=False)
    wv = nc.declare_dram_parameter("wv", [128, DMC, KVH * HD], f16, isOutput=False)
    wo = nc.declare_dram_parameter("wo", [128, NDT, QH, DT], f16, isOutput=False)
    cc = nc.declare_dram_parameter("cc", [HD, S], f32, isOutput=False)
    ssn = nc.declare_dram_parameter("ssn", [HD, S], f32, isOutput=False)
    dmask = nc.declare_dram_parameter("dmask", [SC, G * QT], f16, isOutput=False)
    out = nc.declare_dram_parameter("out", [S, D], f16, isOutput=True)

    with TileContext(nc) as tc, contextlib.ExitStack() as top:
        singles = top.enter_context(tc.tile_pool(name="singles", bufs=1))
        hi_pool = top.enter_context(tc.tile_pool(name="hip", bufs=2))
        wq_pool = top.enter_context(tc.tile_pool(name="wqp", bufs=2))
        wo_pool = top.enter_context(tc.tile_pool(name="wop", bufs=2))
        qt_pool = top.enter_context(tc.tile_pool(name="qtp", bufs=2))
        ctx_pool = top.enter_context(tc.tile_pool(name="ctxp", bufs=2))
        rope_pool = top.enter_context(tc.tile_pool(name="ropep", bufs=2))
        pt_pool = top.enter_context(tc.tile_pool(name="ptp", bufs=6))
        acc_pool = top.enter_context(tc.tile_pool(name="accp", bufs=2))
        misc_pool = top.enter_context(tc.tile_pool(name="miscp", bufs=2))
        o_pool = top.enter_context(tc.tile_pool(name="op", bufs=4))
        ps_mm = top.enter_context(tc.tile_pool(name="ps_mm", bufs=2, space="PSUM"))
        ps_po = top.enter_context(tc.tile_pool(name="ps_po", bufs=2, space="PSUM"))
        ps_s = top.enter_context(tc.tile_pool(name="ps_s", bufs=2, space="PSUM"))
        ps_ctx = top.enter_context(tc.tile_pool(name="ps_ctx", bufs=1, space="PSUM"))
        ps_pb = top.enter_context(tc.tile_pool(name="ps_pb", bufs=1, space="PSUM"))

        wk_sb = singles.tile([128, DMC, KVH * HD], f16)
        nc.sync.dma_start(out=wk_sb, in_=wk[:, :, :])
        wv_sb = singles.tile([128, DMC, KVH * HD], f16)
        nc.sync.dma_start(out=wv_sb, in_=wv[:, :, :])
        # cc/ssn are DMA'd per s-tile slice inside the loop (keeps the first
        # projection matmuls off the critical path of these bulk loads)
        cc_sb = singles.tile([HD, S], f32)
        ssn_sb = singles.tile([HD, S], f32)
        dm_sb = singles.tile([SC, G * QT], f16)
        nc.sync.dma_start(out=dm_sb, in_=dmask[:, :])
        kT_sb = singles.tile([128, KVH, S], f16)
        v_sb = singles.tile([128, NSC, KVH, HD], f16)
        ones_p = singles.tile([128, 1], f16)
        nc.vector.memset(ones_p, 1.0)
        ones_r = singles.tile([1, 128], f32)
        nc.vector.memset(ones_r, 1.0)
        nbias = singles.tile([128, 1], f32)
        nc.vector.memset(nbias, -4.0)

        def rope(dst, psum, ssl):
            """dst(f16) = neox-rope(psum) using cc and sign-folded ssn."""
            t1 = rope_pool.tile([HD, QT], f32, name="t1")
            t2 = rope_pool.tile([HD, QT], f32, name="t2")
            nc.vector.tensor_mul(t1, psum, cc_sb[:, ssl])
            nc.vector.tensor_mul(t2[:64], psum[64:], ssn_sb[:64, ssl])
            nc.vector.tensor_mul(t2[64:], psum[:64], ssn_sb[64:, ssl])
            nc.vector.tensor_add(dst, t1, t2)

        def emit_D(dst_st, dctx):
            """Output projection for s-tile dst_st from its ctx tile."""
            for dt in range(NDT):
                wot = wo_pool.tile([128, QH, DT], f16, name="wot")
                nc.sync.dma_start(out=wot, in_=wo[:, dt])
                for blk in range(QT // SC):
                    po = ps_po.tile([SC, DT], f32, name="po")
                    for h in range(QH):
                        nc.tensor.matmul(
                            po,
                            dctx[:, h, blk * SC : (blk + 1) * SC],
                            wot[:, h, :],
                            start=(h == 0),
                            stop=(h == QH - 1),
                        )
                    osb = o_pool.tile([SC, DT], f16, name="osb")
                    nc.scalar.copy(osb, po)
                    r0 = dst_st * QT + blk * SC
                    nc.sync.dma_start(
                        out=out[r0 : r0 + SC, dt * DT : (dt + 1) * DT], in_=osb
                    )

        prev_ctx = None
        for st in range(NQT):
            ssl = slice(st * QT, (st + 1) * QT)

            slabs = []
            for hh in range(2):
                slab = hi_pool.tile([128, HALF, QT], f16, name="slab")
                nc.sync.dma_start(out=slab, in_=hi[:, st, hh])
                slabs.append(slab)
            nc.sync.dma_start(out=cc_sb[:, ssl], in_=cc[:, ssl])
            nc.sync.dma_start(out=ssn_sb[:, ssl], in_=ssn[:, ssl])

            # ---- K projection (+rope) into resident kT_sb ----
            for kv in range(KVH):
                pk = ps_mm.tile([128, QT], f32, name="mm")
                for c in range(DMC):
                    nc.tensor.matmul(
                        pk,
                        wk_sb[:, c, kv * HD : (kv + 1) * HD],
                        slabs[c // HALF][:, c % HALF, :],
                        start=(c == 0),
                        stop=(c == DMC - 1),
                    )
                rope(kT_sb[:, kv, ssl], pk, ssl)

            # ---- V projection, direct [seq, kv*HD] orientation ----
            for blk in range(QT // SC):
                pv = ps_mm.tile([128, KVH * HD], f32, name="mm")
                for c in range(DMC):
                    nc.tensor.matmul(
                        pv,
                        slabs[c // HALF][:, c % HALF, blk * SC : (blk + 1) * SC],
                        wv_sb[:, c, :],
                        start=(c == 0),
                        stop=(c == DMC - 1),
                    )
                nc.scalar.copy(v_sb[:, st * (QT // SC) + blk, :, :], pv)

            # ---- Q projection (+rope), wq streamed per head ----
            qt_t = qt_pool.tile([128, QH, QT], f16, name="qt")
            for h in range(QH):
                wqh = wq_pool.tile([128, DMC, 128], f16, name="wqh")
                nc.sync.dma_start(out=wqh, in_=wq[:, h])
                pq = ps_mm.tile([128, QT], f32, name="mm")
                for c in range(DMC):
                    nc.tensor.matmul(
                        pq,
                        wqh[:, c, :],
                        slabs[c // HALF][:, c % HALF, :],
                        start=(c == 0),
                        stop=(c == DMC - 1),
                    )
                rope(qt_t[:, h, :], pq, ssl)

            # ---- attention for this query tile ----
            ctx_t = ctx_pool.tile([128, QH, QT], f16, name="ctx")
            nk = G * (st + 1)
            for h in range(QH):
                kv = h // G
                pctx = ps_ctx.tile([128, QT], f32, name="cx")
                acc = acc_pool.tile([SC, QT], f16, name="acc")
                for i in range(nk):
                    # diagonal chunk t covers keys [i*SC, i*SC+SC); queries
                    # below t*SC are fully masked there -> narrow all work to
                    # the live query range [q0, QT)
                    t = i - G * st
                    q0 = t * SC if t > 0 else 0
                    qsl = slice(q0, QT)
                    pss = ps_s.tile([SC, QT], f32, name="ss")
                    nc.tensor.matmul(
                        pss[:, qsl],
                        kT_sb[:, kv, i * SC : (i + 1) * SC],
                        qt_t[:, h, qsl],
                        start=True,
                        stop=True,
                    )
                    pt = pt_pool.tile([SC, QT], f16, name="pt")
                    # bias -4 keeps exp inside fp16 range for extreme score
                    # tails (overflow at s*scale > 15.1 instead of 11.1); the
                    # e^-4 factor cancels exactly in the softmax normalization.
                    nc.scalar.activation(
                        pt[:, qsl], pss[:, qsl], AF.Exp, scale=_SCALE, bias=nbias
                    )
                    if t >= 0:
                        nc.vector.tensor_mul(
                            pt[:, qsl], pt[:, qsl], dm_sb[:, t * QT + q0 : (t + 1) * QT]
                        )
                    if i == 0:
                        nc.vector.tensor_copy(acc, pt)
                    else:
                        nc.vector.tensor_add(acc[:, qsl], acc[:, qsl], pt[:, qsl])
                    nc.tensor.matmul(
                        pctx[:, qsl],
                        v_sb[:, i, kv, :],
                        pt[:, qsl],
                        start=(i == 0),
                        stop=(i == nk - 1),
                    )
                pred = ps_pb.tile([1, QT], f32, name="pb")
                nc.tensor.matmul(pred, ones_p, acc, start=True, stop=True)
                # 1/x via exp(-log(x)) on ScalarE (~2 ULP each; the DVE
                # reciprocal is ~4us per call and the fast custom-DVE variant
                # does not encode on this walrus build)
                ltmp = misc_pool.tile([1, QT], f32, name="ltmp")
                nc.scalar.activation(ltmp, pred, AF.Ln)
                recip32 = misc_pool.tile([1, QT], f32, name="recip32")
                nc.scalar.activation(recip32, ltmp, AF.Exp, scale=-1.0)
                pbc = ps_pb.tile([128, QT], f32, name="pb")
                nc.tensor.matmul(pbc, ones_r, recip32, start=True, stop=True)
                bc = misc_pool.tile([128, QT], f32, name="bc")
                nc.scalar.copy(bc, pbc)
                nc.vector.tensor_mul(ctx_t[:, h, :], pctx, bc)

            # ---- output projection, deferred by one s-tile: D(st-1) is
            # guaranteed-ready tensor filler for attention(st)'s exp stalls
            # (D(st) would only become ready near the end of attention(st))
            if prev_ctx is not None:
                emit_D(st - 1, prev_ctx)
            prev_ctx = ctx_t

        emit_D(NQT - 1, prev_ctx)

    _legalize_waits(nc)
    return nc


_NC_CACHE = {}
_last_exec_ns = None


def _get_nc():
    if "nc" not in _NC_CACHE:
        _NC_CACHE["nc"] = _build_nc()
    return _NC_CACHE["nc"]


# ---------------------------------------------------------------------------
# Optional NTFF profiling hook (used by the local test harness via
# KERNEL_TRACE=1; grading path leaves it off)
# ---------------------------------------------------------------------------
def _install_ntff_hook(so_path="/opt/axon/libaxon_pjrt.so"):
    if "antenv.axon_hooks" in sys.modules:
        return
    try:
        lib = ctypes.CDLL(so_path)
    except OSError:
        lib = None
    if lib is None or not hasattr(lib, "axon_start_nrt_profile"):
        hook = None
    else:
        lib.axon_start_nrt_profile.argtypes = [
            ctypes.POINTER(ctypes.c_int64),
            ctypes.c_size_t,
        ]
        lib.axon_start_nrt_profile.restype = ctypes.c_int64
        lib.axon_stop_nrt_profile.argtypes = [ctypes.c_char_p]
        lib.axon_stop_nrt_profile.restype = ctypes.c_int64

        @contextlib.contextmanager
        def hook(output_dir, device_ids):
            import jax

            jax.devices()
            if device_ids:
                ids = (ctypes.c_int64 * len(device_ids))(*device_ids)
                rc = lib.axon_start_nrt_profile(ids, len(device_ids))
            else:
                rc = lib.axon_start_nrt_profile(None, 0)
            if rc != 0:
                raise RuntimeError(f"axon_start_nrt_profile rc={rc}")
            try:
                yield
            finally:
                n = lib.axon_stop_nrt_profile(str(output_dir).encode())
                print(f"ntff profile: {n} file(s) -> {output_dir}", file=sys.stderr)

    mod = types.ModuleType("antenv.axon_hooks")
    mod.get_axon_ntff_profile_hook = lambda: hook
    sys.modules["antenv.axon_hooks"] = mod


# ---------------------------------------------------------------------------
# Host entry point
# ---------------------------------------------------------------------------
def kernel(hidden_states, position_ids, attention_mask, Wq, Wk, Wv, Wo):
    global _last_exec_ns
    from concourse import bass_utils

    hidden_states = np.asarray(hidden_states, dtype=np.float32)
    position_ids = np.asarray(position_ids)
    attention_mask = np.asarray(attention_mask)
    Wq = np.asarray(Wq, dtype=np.float32)
    Wk = np.asarray(Wk, dtype=np.float32)
    Wv = np.asarray(Wv, dtype=np.float32)
    Wo = np.asarray(Wo, dtype=np.float32)

    if not np.all(np.asarray(attention_mask) > 0):
        # Spec guarantees an all-ones mask; fall back to a host reference
        # implementation for the general case rather than mis-computing.
        return _host_reference(
            hidden_states, position_ids, attention_mask, Wq, Wk, Wv, Wo
        )

    # rope tables per batch: cc = [cos; cos], ssn = [-sin; sin]  (f32 [HD, S])
    half = HD // 2
    inv_freq = 1.0 / (THETA ** (np.arange(0, half, dtype=np.float32) / half))
    ccs, ssns = [], []
    for b in range(B):
        freqs = position_ids[b].astype(np.float32)[:, None] * inv_freq[None, :]
        cosT = np.cos(freqs).T.astype(np.float32)  # [64, S]
        sinT = np.sin(freqs).T.astype(np.float32)
        ccs.append(np.ascontiguousarray(np.concatenate([cosT, cosT], axis=0)))
        ssns.append(np.ascontiguousarray(np.concatenate([-sinT, sinT], axis=0)))

    # multiplicative causal masks for the diagonal blocks: block t in [0, G):
    # dmask[kk, t*QT + qq] = 1 if qq >= t*SC + kk else 0
    kk = np.arange(SC)[:, None]
    qq = np.arange(QT)[None, :]
    dmask = np.concatenate(
        [
            np.where(qq >= t * SC + kk, 1.0, 0.0).astype(np.float16)
            for t in range(G)
        ],
        axis=1,
    )
    dmask = np.ascontiguousarray(dmask)

    # hidden: [p, st, half, c_local, x]  (d = c*128 + p, s = st*QT + x)
    his = []
    for b in range(B):
        hiT = hidden_states[b].T.astype(np.float16)  # [D, S]
        t = hiT.reshape(DMC, 128, NQT, QT).transpose(1, 2, 0, 3)  # [p, st, c, x]
        his.append(np.ascontiguousarray(t.reshape(128, NQT, 2, HALF, QT)))

    in_maps = []
    for c in range(NCORES):
        b = c // KV_SHARDS
        m = c % KV_SHARDS
        FQ = QH * HD
        FKV = KVH * HD
        wq_s = Wq[:, m * FQ : (m + 1) * FQ].astype(np.float16)
        wq_pre = np.ascontiguousarray(
            wq_s.reshape(DMC, 128, QH, HD).transpose(1, 2, 0, 3)
        )  # [p, h, c, x]
        wk_s = Wk[:, m * FKV : (m + 1) * FKV].astype(np.float16)
        wk_pre = np.ascontiguousarray(
            wk_s.reshape(DMC, 128, FKV).transpose(1, 0, 2)
        )  # [p, c, kv*HD]
        wv_s = Wv[:, m * FKV : (m + 1) * FKV].astype(np.float16)
        wv_pre = np.ascontiguousarray(
            wv_s.reshape(DMC, 128, FKV).transpose(1, 0, 2)
        )
        wo_s = Wo[m * FQ : (m + 1) * FQ, :].astype(np.float16)
        wo_pre = np.ascontiguousarray(
            wo_s.reshape(QH, 128, NDT, DT).transpose(1, 2, 0, 3)
        )  # [p, dt, h, x]
        in_maps.append(
            {
                "hi": his[b],
                "wq": wq_pre,
                "wk": wk_pre,
                "wv": wv_pre,
                "wo": wo_pre,
                "cc": ccs[b],
                "ssn": ssns[b],
                "dmask": dmask,
            }
        )

    nc = _get_nc()
    trace = os.environ.get("KERNEL_TRACE", "") == "1"
    if trace:
        _install_ntff_hook()
        bass_utils.upload_artifacts = lambda tmpdir: f"local:{tmpdir}"
    res = bass_utils.run_bass_kernel_spmd(
        nc, in_maps, list(range(NCORES)), trace=trace
    )
    _last_exec_ns = res.exec_time_ns

    out = np.zeros((B, S, D), dtype=np.float32)
    for c in range(NCORES):
        out[c // KV_SHARDS] += np.asarray(res.results[c]["out"], dtype=np.float32)
    return out


def _host_reference(hidden_states, position_ids, attention_mask, Wq, Wk, Wv, Wo):
    """Numpy fallback for inputs outside the spec's guarantees."""
    q = (hidden_states @ Wq).reshape(B, S, H, HD)
    k = (hidden_states @ Wk).reshape(B, S, HKV, HD)
    v = (hidden_states @ Wv).reshape(B, S, HKV, HD)

    half = HD // 2
    inv_freq = 1.0 / (THETA ** (np.arange(0, half, dtype=np.float32) / half))
    freqs = position_ids.astype(np.float32)[..., None] * inv_freq
    cos = np.cos(freqs)[:, :, None, :]
    sin = np.sin(freqs)[:, :, None, :]

    def rope(x):
        x1, x2 = x[..., :half], x[..., half:]
        return np.concatenate([x1 * cos - x2 * sin, x2 * cos + x1 * sin], axis=-1)

    q, k = rope(q), rope(k)
    qg = q.reshape(B, S, HKV, G, HD)
    scores = np.einsum("bqhgd,bkhd->bhgqk", qg, k) * (HD**-0.5)
    causal = np.tril(np.ones((S, S), bool))
    mask = causal[None, None, None] & (attention_mask[:, None, None, None, :] > 0)
    scores = np.where(mask, scores, np.finfo(np.float32).min)
    scores = scores - scores.max(axis=-1, keepdims=True)
    probs = np.exp(scores)
    probs = probs / probs.sum(axis=-1, keepdims=True)
    ctx = np.einsum("bhgqk,bkhd->bqhgd", probs, v).reshape(B, S, H * HD)
    return (ctx @ Wo).astype(np.float32)


# revision 25
# speedup vs baseline: 2.1761x; 1.0657x over previous
"""Trainium2 Bass kernel for nn_ExaoneAttention (dense transformer attention).

Full-input contract: kernel(**inputs) takes the unsharded inputs and returns
the full [B, S, D] output. Internally shards across 8 NeuronCores:
2-way data parallel over batch x 4-way tensor parallel over kv heads
(2 kv heads = 8 query heads per core). Each core computes a partial
output through its Wo row-slice; the host sums the 4 partials per batch.

v2 design (vs the staged f32r baseline):
- fp16 operands everywhere (PE full rate + FWL weight-load hiding, which
  f32r disables; quantization noise ~2^-11 stays well inside the 2e-2 gate).
- Single fused pipeline per 512-query s-tile: QKV projection -> rope ->
  causal attention -> output projection, all SBUF-resident (no DRAM
  staging round trips). K/V accumulate into resident SBUF tiles; the Tile
  scheduler overlaps proj(st+1) matmuls into attention(st)'s exp stalls.
- V is projected directly in [seq, head_dim] orientation (hidden chunk as
  the stationary operand) so no PE transposes are needed.
- Causal masking is a multiplicative 0/1 fp16 mask applied after exp (2x
  DVE rate); softmax denominator accumulates in fp16 (<=16 adds, then an
  exact f32 ones-matmul partition reduce); reciprocal via the fast DVE
  approximation (~18 bits), broadcast back over partitions with a rank-1
  matmul.
"""

import contextlib
import ctypes
import os
import sys
import types

import numpy as np

# ---------------------------------------------------------------------------
# Problem constants (hardcoded per contract)
# ---------------------------------------------------------------------------
B, S, D = 2, 2048, 4096
H, HKV, HD = 32, 8, 128
G = H // HKV
THETA = 10000.0

NCORES = 8
BAT_SHARDS = 2
KV_SHARDS = 4
KVH = HKV // KV_SHARDS  # kv heads per core = 2
QH = KVH * G  # q heads per core = 8
DMC = D // 128  # 32 model-dim chunks
HALF = DMC // 2  # chunks per hidden slab

QT = 512  # query tile
NQT = S // QT  # 4
SC = 128  # key chunk
NSC = S // SC  # 16
DT = 512  # output d tile
NDT = D // DT  # 8

_SCALE = float(HD) ** -0.5


# ---------------------------------------------------------------------------
# Wait-count legalization: this walrus build rejects instructions carrying
# more than a small number of sync waits (fused fp32/fp32r matmul: >1;
# drain: >4). Hoist excess waits onto standalone NoOps on the same engine
# immediately before the offending instruction; AND-semantics are preserved
# by sequential same-engine execution.
# ---------------------------------------------------------------------------
def _legalize_waits(nc):
    import bass_rust
    import concourse.mybir as mybir

    counter = 0
    for f in nc.m.functions:
        for bb in f.blocks:
            il = bb.instructions
            i = 0
            while i < len(il):
                ins = il[i]
                si = ins.sync_info
                if si is None or len(si.on_wait) <= 1:
                    i += 1
                    continue
                waits = list(si.on_wait)
                pos = i
                for w in waits[1:]:
                    counter += 1
                    nop = mybir.InstNoOp(name=f"lgw-{counter}", ins=[], outs=[])
                    nop.engine = ins.engine
                    nop.sync_info = bass_rust.SyncInfo(on_wait=[w], on_update=[])
                    il.insert(pos, nop)
                    pos += 1
                    i += 1
                ins.sync_info = bass_rust.SyncInfo(
                    on_wait=waits[:1], on_update=list(si.on_update)
                )
                i += 1
    return counter


# ---------------------------------------------------------------------------
# Bass kernel builder (per-core program; same program on all 8 cores)
# ---------------------------------------------------------------------------
def _build_nc():
    import concourse.bass as bass
    import concourse.mybir as mybir
    from concourse.tile import TileContext

    f32 = mybir.dt.float32
    f16 = mybir.dt.float16
    AF = mybir.ActivationFunctionType

    nc = bass.Bass()

    # host-prearranged layouts (partition dim first everywhere)
    hi = nc.declare_dram_parameter("hi", [128, NQT, 2, HALF, QT], f16, isOutput=False)
    wq = nc.declare_dram_parameter("wq", [128, QH, DMC, 128], f16, isOutput=False)
    wk = nc.declare_dram_parameter("wk", [128, DMC, KVH * HD], f16, isOutput=False)
    wv = nc.declare_dram_parameter("wv", [128, DMC, KVH * HD], f16, isOutput=False)
    wo = nc.declare_dram_parameter("wo", [128, NDT, QH, DT], f16, isOutput=False)
    cc = nc.declare_dram_parameter("cc", [HD, S], f32, isOutput=False)
    ssn = nc.declare_dram_parameter("ssn", [HD, S], f32, isOutput=False)
    dmask = nc.declare_dram_parameter("dmask", [SC, G * QT], f16, isOutput=False)
    out = nc.declare_dram_parameter("out", [S, D], f16, isOutput=True)

    with TileContext(nc) as tc, contextlib.ExitStack() as top:
        singles = top.enter_context(tc.tile_pool(name="singles", bufs=1))
        hi_pool = top.enter_context(tc.tile_pool(name="hip", bufs=2))
        wq_pool = top.enter_context(tc.tile_pool(name="wqp", bufs=2))
        wo_pool = top.enter_context(tc.tile_pool(name="wop", bufs=2))
        qt_pool = top.enter_context(tc.tile_pool(name="qtp", bufs=2))
        ctx_pool = top.enter_context(tc.tile_pool(name="ctxp", bufs=2))
        rope_pool = top.enter_context(tc.tile_pool(name="ropep", bufs=2))
        pt_pool = top.enter_context(tc.tile_pool(name="ptp", bufs=6))
        acc_pool = top.enter_context(tc.tile_pool(name="accp", bufs=2))
        misc_pool = top.enter_context(tc.tile_pool(name="miscp", bufs=2))
        o_pool = top.enter_context(tc.tile_pool(name="op", bufs=4))
        ps_mm = top.enter_context(tc.tile_pool(name="ps_mm", bufs=2, space="PSUM"))
        ps_po = top.enter_context(tc.tile_pool(name="ps_po", bufs=2, space="PSUM"))
        ps_s = top.enter_context(tc.tile_pool(name="ps_s", bufs=2, space="PSUM"))
        ps_ctx = top.enter_context(tc.tile_pool(name="ps_ctx", bufs=1, space="PSUM"))
        ps_pb = top.enter_context(tc.tile_pool(name="ps_pb", bufs=1, space="PSUM"))

        # split the startup-gating loads so the first projection matmuls wait
        # only on the chunk range they actually read (subtile deps)
        wk_sb = singles.tile([128, DMC, KVH * HD], f16)
        nc.sync.dma_start(out=wk_sb[:, :HALF], in_=wk[:, :HALF])
        nc.sync.dma_start(out=wk_sb[:, HALF:], in_=wk[:, HALF:])
        wv_sb = singles.tile([128, DMC, KVH * HD], f16)
        nc.sync.dma_start(out=wv_sb[:, :HALF], in_=wv[:, :HALF])
        nc.sync.dma_start(out=wv_sb[:, HALF:], in_=wv[:, HALF:])
        # cc/ssn are DMA'd per s-tile slice inside the loop (keeps the first
        # projection matmuls off the critical path of these bulk loads)
        cc_sb = singles.tile([HD, S], f32)
        ssn_sb = singles.tile([HD, S], f32)
        dm_sb = singles.tile([SC, G * QT], f16)
        nc.sync.dma_start(out=dm_sb, in_=dmask[:, :])
        kT_sb = singles.tile([128, KVH, S], f16)
        v_sb = singles.tile([128, NSC, KVH, HD], f16)
        ones128 = singles.tile([128, 128], f16)
        nc.vector.memset(ones128, 1.0)
        nbias = singles.tile([128, 1], f32)
        nc.vector.memset(nbias, -4.0)

        def rope(dst, psum, ssl):
            """dst(f16) = neox-rope(psum) using cc and sign-folded ssn."""
            t1 = rope_pool.tile([HD, QT], f32, name="t1")
            t2 = rope_pool.tile([HD, QT], f32, name="t2")
            nc.vector.tensor_mul(t1, psum, cc_sb[:, ssl])
            nc.vector.tensor_mul(t2[:64], psum[64:], ssn_sb[:64, ssl])
            nc.vector.tensor_mul(t2[64:], psum[:64], ssn_sb[64:, ssl])
            nc.vector.tensor_add(dst, t1, t2)

        def emit_D(dst_st, dctx):
            """Output projection for s-tile dst_st from its ctx tile."""
            for dt in range(NDT):
                wot = wo_pool.tile([128, QH, DT], f16, name="wot")
                nc.sync.dma_start(out=wot, in_=wo[:, dt])
                for blk in range(QT // SC):
                    po = ps_po.tile([SC, DT], f32, name="po")
                    for h in range(QH):
                        nc.tensor.matmul(
                            po,
                            dctx[:, h, blk * SC : (blk + 1) * SC],
                            wot[:, h, :],
                            start=(h == 0),
                            stop=(h == QH - 1),
                        )
                    osb = o_pool.tile([SC, DT], f16, name="osb")
                    nc.scalar.copy(osb, po)
                    r0 = dst_st * QT + blk * SC
                    nc.sync.dma_start(
                        out=out[r0 : r0 + SC, dt * DT : (dt + 1) * DT], in_=osb
                    )

        prev_ctx = None
        for st in range(NQT):
            ssl = slice(st * QT, (st + 1) * QT)

            slabs = []
            for hh in range(2):
                slab = hi_pool.tile([128, HALF, QT], f16, name="slab")
                nc.sync.dma_start(out=slab[:, : HALF // 2], in_=hi[:, st, hh, : HALF // 2])
                nc.sync.dma_start(out=slab[:, HALF // 2 :], in_=hi[:, st, hh, HALF // 2 :])
                slabs.append(slab)
            nc.sync.dma_start(out=cc_sb[:, ssl], in_=cc[:, ssl])
            nc.sync.dma_start(out=ssn_sb[:, ssl], in_=ssn[:, ssl])

            # ---- K projection (+rope) into resident kT_sb ----
            for kv in range(KVH):
                pk = ps_mm.tile([128, QT], f32, name="mm")
                for c in range(DMC):
                    nc.tensor.matmul(
                        pk,
                        wk_sb[:, c, kv * HD : (kv + 1) * HD],
                        slabs[c // HALF][:, c % HALF, :],
                        start=(c == 0),
                        stop=(c == DMC - 1),
                    )
                rope(kT_sb[:, kv, ssl], pk, ssl)

            # ---- V projection, direct [seq, kv*HD] orientation ----
            for blk in range(QT // SC):
                pv = ps_mm.tile([128, KVH * HD], f32, name="mm")
                for c in range(DMC):
                    nc.tensor.matmul(
                        pv,
                        slabs[c // HALF][:, c % HALF, blk * SC : (blk + 1) * SC],
                        wv_sb[:, c, :],
                        start=(c == 0),
                        stop=(c == DMC - 1),
                    )
                nc.scalar.copy(v_sb[:, st * (QT // SC) + blk, :, :], pv)

            # ---- Q projection (+rope), wq streamed per head ----
            qt_t = qt_pool.tile([128, QH, QT], f16, name="qt")
            for h in range(QH):
                wqh = wq_pool.tile([128, DMC, 128], f16, name="wqh")
                nc.sync.dma_start(out=wqh, in_=wq[:, h])
                pq = ps_mm.tile([128, QT], f32, name="mm")
                for c in range(DMC):
                    nc.tensor.matmul(
                        pq,
                        wqh[:, c, :],
                        slabs[c // HALF][:, c % HALF, :],
                        start=(c == 0),
                        stop=(c == DMC - 1),
                    )
                rope(qt_t[:, h, :], pq, ssl)

            # ---- attention for this query tile ----
            ctx_t = ctx_pool.tile([128, QH, QT], f16, name="ctx")
            nk = G * (st + 1)
            for h in range(QH):
                kv = h // G
                pctx = ps_ctx.tile([128, QT], f32, name="cx")
                acc = acc_pool.tile([SC, QT], f16, name="acc")
                for i in range(nk):
                    # diagonal chunk t covers keys [i*SC, i*SC+SC); queries
                    # below t*SC are fully masked there -> narrow all work to
                    # the live query range [q0, QT)
                    t = i - G * st
                    q0 = t * SC if t > 0 else 0
                    qsl = slice(q0, QT)
                    pss = ps_s.tile([SC, QT], f32, name="ss")
                    nc.tensor.matmul(
                        pss[:, qsl],
                        kT_sb[:, kv, i * SC : (i + 1) * SC],
                        qt_t[:, h, qsl],
                        start=True,
                        stop=True,
                    )
                    pt = pt_pool.tile([SC, QT], f16, name="pt")
                    # bias -4 keeps exp inside fp16 range for extreme score
                    # tails (overflow at s*scale > 15.1 instead of 11.1); the
                    # e^-4 factor cancels exactly in the softmax normalization.
                    nc.scalar.activation(
                        pt[:, qsl], pss[:, qsl], AF.Exp, scale=_SCALE, bias=nbias
                    )
                    if t >= 0:
                        nc.vector.tensor_mul(
                            pt[:, qsl], pt[:, qsl], dm_sb[:, t * QT + q0 : (t + 1) * QT]
                        )
                    if i == 0:
                        nc.vector.tensor_copy(acc, pt)
                    else:
                        nc.vector.tensor_add(acc[:, qsl], acc[:, qsl], pt[:, qsl])
                    nc.tensor.matmul(
                        pctx[:, qsl],
                        v_sb[:, i, kv, :],
                        pt[:, qsl],
                        start=(i == 0),
                        stop=(i == nk - 1),
                    )
                # all-ones stationary: every output row of pred128 is the
                # partition-sum of acc -> reduce AND broadcast in one full-rate
                # matmul. 1/x then via exp(-ln(x)) on ScalarE (~2 ULP each; the
                # DVE reciprocal is ~4us per call and the custom-DVE fast
                # variant does not encode on this walrus build).
                pred128 = ps_pb.tile([128, QT], f32, name="pb")
                nc.tensor.matmul(pred128, ones128, acc, start=True, stop=True)
                ltmp = misc_pool.tile([128, QT], f32, name="ltmp")
                nc.scalar.activation(ltmp, pred128, AF.Ln)
                bc = misc_pool.tile([128, QT], f32, name="bc")
                nc.scalar.activation(bc, ltmp, AF.Exp, scale=-1.0)
                nc.vector.tensor_mul(ctx_t[:, h, :], pctx, bc)

            # ---- output projection, deferred by one s-tile: D(st-1) is
            # guaranteed-ready tensor filler for attention(st)'s exp stalls
            # (D(st) would only become ready near the end of attention(st))
            if prev_ctx is not None:
                emit_D(st - 1, prev_ctx)
            prev_ctx = ctx_t

        emit_D(NQT - 1, prev_ctx)

    _legalize_waits(nc)
    return nc


_NC_CACHE = {}
_last_exec_ns = None


def _get_nc():
    if "nc" not in _NC_CACHE:
        _NC_CACHE["nc"] = _build_nc()
    return _NC_CACHE["nc"]


# ---------------------------------------------------------------------------
# Optional NTFF profiling hook (used by the local test harness via
# KERNEL_TRACE=1; grading path leaves it off)
# ---------------------------------------------------------------------------
def _install_ntff_hook(so_path="/opt/axon/libaxon_pjrt.so"):
    if "antenv.axon_hooks" in sys.modules:
        return
    try:
        lib = ctypes.CDLL(so_path)
    except OSError:
        lib = None
    if lib is None or not hasattr(lib, "axon_start_nrt_profile"):
        hook = None
    else:
        lib.axon_start_nrt_profile.argtypes = [
            ctypes.POINTER(ctypes.c_int64),
            ctypes.c_size_t,
        ]
        lib.axon_start_nrt_profile.restype = ctypes.c_int64
        lib.axon_stop_nrt_profile.argtypes = [ctypes.c_char_p]
        lib.axon_stop_nrt_profile.restype = ctypes.c_int64

        @contextlib.contextmanager
        def hook(output_dir, device_ids):
            import jax

            jax.devices()
            if device_ids:
                ids = (ctypes.c_int64 * len(device_ids))(*device_ids)
                rc = lib.axon_start_nrt_profile(ids, len(device_ids))
            else:
                rc = lib.axon_start_nrt_profile(None, 0)
            if rc != 0:
                raise RuntimeError(f"axon_start_nrt_profile rc={rc}")
            try:
                yield
            finally:
                n = lib.axon_stop_nrt_profile(str(output_dir).encode())
                print(f"ntff profile: {n} file(s) -> {output_dir}", file=sys.stderr)

    mod = types.ModuleType("antenv.axon_hooks")
    mod.get_axon_ntff_profile_hook = lambda: hook
    sys.modules["antenv.axon_hooks"] = mod


# ---------------------------------------------------------------------------
# Host entry point
# ---------------------------------------------------------------------------
def kernel(hidden_states, position_ids, attention_mask, Wq, Wk, Wv, Wo):
    global _last_exec_ns
    from concourse import bass_utils

    hidden_states = np.asarray(hidden_states, dtype=np.float32)
    position_ids = np.asarray(position_ids)
    attention_mask = np.asarray(attention_mask)
    Wq = np.asarray(Wq, dtype=np.float32)
    Wk = np.asarray(Wk, dtype=np.float32)
    Wv = np.asarray(Wv, dtype=np.float32)
    Wo = np.asarray(Wo, dtype=np.float32)

    if not np.all(np.asarray(attention_mask) > 0):
        # Spec guarantees an all-ones mask; fall back to a host reference
        # implementation for the general case rather than mis-computing.
        return _host_reference(
            hidden_states, position_ids, attention_mask, Wq, Wk, Wv, Wo
        )

    # rope tables per batch: cc = [cos; cos], ssn = [-sin; sin]  (f32 [HD, S])
    half = HD // 2
    inv_freq = 1.0 / (THETA ** (np.arange(0, half, dtype=np.float32) / half))
    ccs, ssns = [], []
    for b in range(B):
        freqs = position_ids[b].astype(np.float32)[:, None] * inv_freq[None, :]
        cosT = np.cos(freqs).T.astype(np.float32)  # [64, S]
        sinT = np.sin(freqs).T.astype(np.float32)
        ccs.append(np.ascontiguousarray(np.concatenate([cosT, cosT], axis=0)))
        ssns.append(np.ascontiguousarray(np.concatenate([-sinT, sinT], axis=0)))

    # multiplicative causal masks for the diagonal blocks: block t in [0, G):
    # dmask[kk, t*QT + qq] = 1 if qq >= t*SC + kk else 0
    kk = np.arange(SC)[:, None]
    qq = np.arange(QT)[None, :]
    dmask = np.concatenate(
        [
            np.where(qq >= t * SC + kk, 1.0, 0.0).astype(np.float16)
            for t in range(G)
        ],
        axis=1,
    )
    dmask = np.ascontiguousarray(dmask)

    # hidden: [p, st, half, c_local, x]  (d = c*128 + p, s = st*QT + x)
    his = []
    for b in range(B):
        hiT = hidden_states[b].T.astype(np.float16)  # [D, S]
        t = hiT.reshape(DMC, 128, NQT, QT).transpose(1, 2, 0, 3)  # [p, st, c, x]
        his.append(np.ascontiguousarray(t.reshape(128, NQT, 2, HALF, QT)))

    in_maps = []
    for c in range(NCORES):
        b = c // KV_SHARDS
        m = c % KV_SHARDS
        FQ = QH * HD
        FKV = KVH * HD
        wq_s = Wq[:, m * FQ : (m + 1) * FQ].astype(np.float16)
        wq_pre = np.ascontiguousarray(
            wq_s.reshape(DMC, 128, QH, HD).transpose(1, 2, 0, 3)
        )  # [p, h, c, x]
        wk_s = Wk[:, m * FKV : (m + 1) * FKV].astype(np.float16)
        wk_pre = np.ascontiguousarray(
            wk_s.reshape(DMC, 128, FKV).transpose(1, 0, 2)
        )  # [p, c, kv*HD]
        wv_s = Wv[:, m * FKV : (m + 1) * FKV].astype(np.float16)
        wv_pre = np.ascontiguousarray(
            wv_s.reshape(DMC, 128, FKV).transpose(1, 0, 2)
        )
        wo_s = Wo[m * FQ : (m + 1) * FQ, :].astype(np.float16)
        wo_pre = np.ascontiguousarray(
            wo_s.reshape(QH, 128, NDT, DT).transpose(1, 2, 0, 3)
        )  # [p, dt, h, x]
        in_maps.append(
            {
                "hi": his[b],
                "wq": wq_pre,
                "wk": wk_pre,
                "wv": wv_pre,
                "wo": wo_pre,
                "cc": ccs[b],
                "ssn": ssns[b],
                "dmask": dmask,
            }
        )

    nc = _get_nc()
    trace = os.environ.get("KERNEL_TRACE", "") == "1"
    if trace:
        _install_ntff_hook()
        bass_utils.upload_artifacts = lambda tmpdir: f"local:{tmpdir}"
    res = bass_utils.run_bass_kernel_spmd(
        nc, in_maps, list(range(NCORES)), trace=trace
    )
    _last_exec_ns = res.exec_time_ns

    out = np.zeros((B, S, D), dtype=np.float32)
    for c in range(NCORES):
        out[c // KV_SHARDS] += np.asarray(res.results[c]["out"], dtype=np.float32)
    return out


def _host_reference(hidden_states, position_ids, attention_mask, Wq, Wk, Wv, Wo):
    """Numpy fallback for inputs outside the spec's guarantees."""
    q = (hidden_states @ Wq).reshape(B, S, H, HD)
    k = (hidden_states @ Wk).reshape(B, S, HKV, HD)
    v = (hidden_states @ Wv).reshape(B, S, HKV, HD)

    half = HD // 2
    inv_freq = 1.0 / (THETA ** (np.arange(0, half, dtype=np.float32) / half))
    freqs = position_ids.astype(np.float32)[..., None] * inv_freq
    cos = np.cos(freqs)[:, :, None, :]
    sin = np.sin(freqs)[:, :, None, :]

    def rope(x):
        x1, x2 = x[..., :half], x[..., half:]
        return np.concatenate([x1 * cos - x2 * sin, x2 * cos + x1 * sin], axis=-1)

    q, k = rope(q), rope(k)
    qg = q.reshape(B, S, HKV, G, HD)
    scores = np.einsum("bqhgd,bkhd->bhgqk", qg, k) * (HD**-0.5)
    causal = np.tril(np.ones((S, S), bool))
    mask = causal[None, None, None] & (attention_mask[:, None, None, None, :] > 0)
    scores = np.where(mask, scores, np.finfo(np.float32).min)
    scores = scores - scores.max(axis=-1, keepdims=True)
    probs = np.exp(scores)
    probs = probs / probs.sum(axis=-1, keepdims=True)
    ctx = np.einsum("bhgqk,bkhd->bqhgd", probs, v).reshape(B, S, H * HD)
    return (ctx @ Wo).astype(np.float32)


# revision 27
# speedup vs baseline: 2.1856x; 1.0044x over previous
"""Trainium2 Bass kernel for nn_ExaoneAttention (dense transformer attention).

Full-input contract: kernel(**inputs) takes the unsharded inputs and returns
the full [B, S, D] output. Internally shards across 8 NeuronCores:
2-way data parallel over batch x 4-way tensor parallel over kv heads
(2 kv heads = 8 query heads per core). Each core computes a partial
output through its Wo row-slice; the host sums the 4 partials per batch.

v2 design (vs the staged f32r baseline):
- fp16 operands everywhere (PE full rate + FWL weight-load hiding, which
  f32r disables; quantization noise ~2^-11 stays well inside the 2e-2 gate).
- Single fused pipeline per 512-query s-tile: QKV projection -> rope ->
  causal attention -> output projection, all SBUF-resident (no DRAM
  staging round trips). K/V accumulate into resident SBUF tiles; the Tile
  scheduler overlaps proj(st+1) matmuls into attention(st)'s exp stalls.
- V is projected directly in [seq, head_dim] orientation (hidden chunk as
  the stationary operand) so no PE transposes are needed.
- Causal masking is a multiplicative 0/1 fp16 mask applied after exp (2x
  DVE rate); softmax denominator accumulates in fp16 (<=16 adds, then an
  exact f32 ones-matmul partition reduce); reciprocal via the fast DVE
  approximation (~18 bits), broadcast back over partitions with a rank-1
  matmul.
"""

import contextlib
import ctypes
import os
import sys
import types

import numpy as np

# ---------------------------------------------------------------------------
# Problem constants (hardcoded per contract)
# ---------------------------------------------------------------------------
B, S, D = 2, 2048, 4096
H, HKV, HD = 32, 8, 128
G = H // HKV
THETA = 10000.0

NCORES = 8
BAT_SHARDS = 2
KV_SHARDS = 4
KVH = HKV // KV_SHARDS  # kv heads per core = 2
QH = KVH * G  # q heads per core = 8
DMC = D // 128  # 32 model-dim chunks
HALF = DMC // 2  # chunks per hidden slab

QT = 512  # query tile
NQT = S // QT  # 4
SC = 128  # key chunk
NSC = S // SC  # 16
DT = 512  # output d tile
NDT = D // DT  # 8

_SCALE = float(HD) ** -0.5


# ---------------------------------------------------------------------------
# Wait-count legalization: this walrus build rejects instructions carrying
# more than a small number of sync waits (fused fp32/fp32r matmul: >1;
# drain: >4). Hoist excess waits onto standalone NoOps on the same engine
# immediately before the offending instruction; AND-semantics are preserved
# by sequential same-engine execution.
# ---------------------------------------------------------------------------
def _legalize_waits(nc):
    import bass_rust
    import concourse.mybir as mybir

    counter = 0
    for f in nc.m.functions:
        for bb in f.blocks:
            il = bb.instructions
            i = 0
            while i < len(il):
                ins = il[i]
                si = ins.sync_info
                if si is None or len(si.on_wait) <= 1:
                    i += 1
                    continue
                waits = list(si.on_wait)
                pos = i
                for w in waits[1:]:
                    counter += 1
                    nop = mybir.InstNoOp(name=f"lgw-{counter}", ins=[], outs=[])
                    nop.engine = ins.engine
                    nop.sync_info = bass_rust.SyncInfo(on_wait=[w], on_update=[])
                    il.insert(pos, nop)
                    pos += 1
                    i += 1
                ins.sync_info = bass_rust.SyncInfo(
                    on_wait=waits[:1], on_update=list(si.on_update)
                )
                i += 1
    return counter


# ---------------------------------------------------------------------------
# Bass kernel builder (per-core program; same program on all 8 cores)
# ---------------------------------------------------------------------------
def _build_nc():
    import concourse.bass as bass
    import concourse.mybir as mybir
    from concourse.tile import TileContext

    f32 = mybir.dt.float32
    f16 = mybir.dt.float16
    AF = mybir.ActivationFunctionType

    nc = bass.Bass()

    # host-prearranged layouts (partition dim first everywhere)
    hi = nc.declare_dram_parameter("hi", [128, NQT, 2, HALF, QT], f16, isOutput=False)
    wq = nc.declare_dram_parameter("wq", [128, QH, DMC, 128], f16, isOutput=False)
    wk = nc.declare_dram_parameter("wk", [128, DMC, KVH * HD], f16, isOutput=False)
    wv = nc.declare_dram_parameter("wv", [128, DMC, KVH * HD], f16, isOutput=False)
    wo = nc.declare_dram_parameter("wo", [128, NDT, QH, DT], f16, isOutput=False)
    cc = nc.declare_dram_parameter("cc", [HD, S], f32, isOutput=False)
    ssn = nc.declare_dram_parameter("ssn", [HD, S], f32, isOutput=False)
    dmask = nc.declare_dram_parameter("dmask", [SC, G * QT], f16, isOutput=False)
    out = nc.declare_dram_parameter("out", [S, D], f16, isOutput=True)

    with TileContext(nc) as tc, contextlib.ExitStack() as top:
        singles = top.enter_context(tc.tile_pool(name="singles", bufs=1))
        hi_pool = top.enter_context(tc.tile_pool(name="hip", bufs=2))
        wq_pool = top.enter_context(tc.tile_pool(name="wqp", bufs=2))
        wo_pool = top.enter_context(tc.tile_pool(name="wop", bufs=2))
        qt_pool = top.enter_context(tc.tile_pool(name="qtp", bufs=2))
        ctx_pool = top.enter_context(tc.tile_pool(name="ctxp", bufs=2))
        rope_pool = top.enter_context(tc.tile_pool(name="ropep", bufs=2))
        pt_pool = top.enter_context(tc.tile_pool(name="ptp", bufs=6))
        acc_pool = top.enter_context(tc.tile_pool(name="accp", bufs=2))
        misc_pool = top.enter_context(tc.tile_pool(name="miscp", bufs=2))
        o_pool = top.enter_context(tc.tile_pool(name="op", bufs=4))
        ps_mm = top.enter_context(tc.tile_pool(name="ps_mm", bufs=2, space="PSUM"))
        ps_po = top.enter_context(tc.tile_pool(name="ps_po", bufs=2, space="PSUM"))
        ps_s = top.enter_context(tc.tile_pool(name="ps_s", bufs=2, space="PSUM"))
        ps_ctx = top.enter_context(tc.tile_pool(name="ps_ctx", bufs=1, space="PSUM"))
        ps_pb = top.enter_context(tc.tile_pool(name="ps_pb", bufs=1, space="PSUM"))

        # split the startup-gating loads so the first projection matmuls wait
        # only on the chunk range they actually read (subtile deps)
        wk_sb = singles.tile([128, DMC, KVH * HD], f16)
        for qtr in range(4):
            csl = slice(qtr * (DMC // 4), (qtr + 1) * (DMC // 4))
            nc.sync.dma_start(out=wk_sb[:, csl], in_=wk[:, csl])
        wv_sb = singles.tile([128, DMC, KVH * HD], f16)
        nc.sync.dma_start(out=wv_sb[:, :HALF], in_=wv[:, :HALF])
        nc.sync.dma_start(out=wv_sb[:, HALF:], in_=wv[:, HALF:])
        # cc/ssn are DMA'd per s-tile slice inside the loop (keeps the first
        # projection matmuls off the critical path of these bulk loads)
        cc_sb = singles.tile([HD, S], f32)
        ssn_sb = singles.tile([HD, S], f32)
        dm_sb = singles.tile([SC, G * QT], f16)
        nc.sync.dma_start(out=dm_sb, in_=dmask[:, :])
        kT_sb = singles.tile([128, KVH, S], f16)
        v_sb = singles.tile([128, NSC, KVH, HD], f16)
        ones128 = singles.tile([128, 128], f16)
        nc.vector.memset(ones128, 1.0)
        nbias = singles.tile([128, 1], f32)
        nc.vector.memset(nbias, -4.0)

        def rope(dst, psum, ssl):
            """dst(f16) = neox-rope(psum) using cc and sign-folded ssn."""
            t1 = rope_pool.tile([HD, QT], f32, name="t1")
            t2 = rope_pool.tile([HD, QT], f32, name="t2")
            nc.vector.tensor_mul(t1, psum, cc_sb[:, ssl])
            nc.vector.tensor_mul(t2[:64], psum[64:], ssn_sb[:64, ssl])
            nc.vector.tensor_mul(t2[64:], psum[:64], ssn_sb[64:, ssl])
            nc.vector.tensor_add(dst, t1, t2)

        def emit_D(dst_st, dctx):
            """Output projection for s-tile dst_st from its ctx tile."""
            for dt in range(NDT):
                wot = wo_pool.tile([128, QH, DT], f16, name="wot")
                nc.sync.dma_start(out=wot, in_=wo[:, dt])
                for blk in range(QT // SC):
                    po = ps_po.tile([SC, DT], f32, name="po")
                    for h in range(QH):
                        nc.tensor.matmul(
                            po,
                            dctx[:, h, blk * SC : (blk + 1) * SC],
                            wot[:, h, :],
                            start=(h == 0),
                            stop=(h == QH - 1),
                        )
                    osb = o_pool.tile([SC, DT], f16, name="osb")
                    nc.scalar.copy(osb, po)
                    r0 = dst_st * QT + blk * SC
                    nc.sync.dma_start(
                        out=out[r0 : r0 + SC, dt * DT : (dt + 1) * DT], in_=osb
                    )

        prev_ctx = None
        for st in range(NQT):
            ssl = slice(st * QT, (st + 1) * QT)

            slabs = []
            for hh in range(2):
                slab = hi_pool.tile([128, HALF, QT], f16, name="slab")
                for qtr in range(4):
                    csl = slice(qtr * (HALF // 4), (qtr + 1) * (HALF // 4))
                    nc.sync.dma_start(out=slab[:, csl], in_=hi[:, st, hh, csl])
                slabs.append(slab)
            nc.sync.dma_start(out=cc_sb[:, ssl], in_=cc[:, ssl])
            nc.sync.dma_start(out=ssn_sb[:, ssl], in_=ssn[:, ssl])

            # ---- K projection (+rope) into resident kT_sb ----
            for kv in range(KVH):
                pk = ps_mm.tile([128, QT], f32, name="mm")
                for c in range(DMC):
                    nc.tensor.matmul(
                        pk,
                        wk_sb[:, c, kv * HD : (kv + 1) * HD],
                        slabs[c // HALF][:, c % HALF, :],
                        start=(c == 0),
                        stop=(c == DMC - 1),
                    )
                rope(kT_sb[:, kv, ssl], pk, ssl)

            # ---- V projection, direct [seq, kv*HD] orientation ----
            for blk in range(QT // SC):
                pv = ps_mm.tile([128, KVH * HD], f32, name="mm")
                for c in range(DMC):
                    nc.tensor.matmul(
                        pv,
                        slabs[c // HALF][:, c % HALF, blk * SC : (blk + 1) * SC],
                        wv_sb[:, c, :],
                        start=(c == 0),
                        stop=(c == DMC - 1),
                    )
                nc.scalar.copy(v_sb[:, st * (QT // SC) + blk, :, :], pv)

            # ---- Q projection (+rope), wq streamed per head ----
            qt_t = qt_pool.tile([128, QH, QT], f16, name="qt")
            for h in range(QH):
                wqh = wq_pool.tile([128, DMC, 128], f16, name="wqh")
                nc.sync.dma_start(out=wqh, in_=wq[:, h])
                pq = ps_mm.tile([128, QT], f32, name="mm")
                for c in range(DMC):
                    nc.tensor.matmul(
                        pq,
                        wqh[:, c, :],
                        slabs[c // HALF][:, c % HALF, :],
                        start=(c == 0),
                        stop=(c == DMC - 1),
                    )
                rope(qt_t[:, h, :], pq, ssl)

            # ---- attention for this query tile ----
            ctx_t = ctx_pool.tile([128, QH, QT], f16, name="ctx")
            nk = G * (st + 1)
            for h in range(QH):
                kv = h // G
                pctx = ps_ctx.tile([128, QT], f32, name="cx")
                acc = acc_pool.tile([SC, QT], f16, name="acc")
                for i in range(nk):
                    # diagonal chunk t covers keys [i*SC, i*SC+SC); queries
                    # below t*SC are fully masked there -> narrow all work to
                    # the live query range [q0, QT)
                    t = i - G * st
                    q0 = t * SC if t > 0 else 0
                    qsl = slice(q0, QT)
                    pss = ps_s.tile([SC, QT], f32, name="ss")
                    nc.tensor.matmul(
                        pss[:, qsl],
                        kT_sb[:, kv, i * SC : (i + 1) * SC],
                        qt_t[:, h, qsl],
                        start=True,
                        stop=True,
                    )
                    pt = pt_pool.tile([SC, QT], f16, name="pt")
                    # bias -4 keeps exp inside fp16 range for extreme score
                    # tails (overflow at s*scale > 15.1 instead of 11.1); the
                    # e^-4 factor cancels exactly in the softmax normalization.
                    nc.scalar.activation(
                        pt[:, qsl], pss[:, qsl], AF.Exp, scale=_SCALE, bias=nbias
                    )
                    if t >= 0:
                        nc.vector.tensor_mul(
                            pt[:, qsl], pt[:, qsl], dm_sb[:, t * QT + q0 : (t + 1) * QT]
                        )
                    if i == 0:
                        nc.vector.tensor_copy(acc, pt)
                    else:
                        nc.vector.tensor_add(acc[:, qsl], acc[:, qsl], pt[:, qsl])
                    nc.tensor.matmul(
                        pctx[:, qsl],
                        v_sb[:, i, kv, :],
                        pt[:, qsl],
                        start=(i == 0),
                        stop=(i == nk - 1),
                    )
                # all-ones stationary: every output row of pred128 is the
                # partition-sum of acc -> reduce AND broadcast in one full-rate
                # matmul. 1/x then via exp(-ln(x)) on ScalarE (~2 ULP each; the
                # DVE reciprocal is ~4us per call and the custom-DVE fast
                # variant does not encode on this walrus build).
                pred128 = ps_pb.tile([128, QT], f32, name="pb")
                nc.tensor.matmul(pred128, ones128, acc, start=True, stop=True)
                ltmp = misc_pool.tile([128, QT], f32, name="ltmp")
                nc.scalar.activation(ltmp, pred128, AF.Ln)
                bc = misc_pool.tile([128, QT], f32, name="bc")
                nc.scalar.activation(bc, ltmp, AF.Exp, scale=-1.0)
                nc.vector.tensor_mul(ctx_t[:, h, :], pctx, bc)

            # ---- output projection, deferred by one s-tile: D(st-1) is
            # guaranteed-ready tensor filler for attention(st)'s exp stalls
            # (D(st) would only become ready near the end of attention(st))
            if prev_ctx is not None:
                emit_D(st - 1, prev_ctx)
            prev_ctx = ctx_t

        emit_D(NQT - 1, prev_ctx)

    _legalize_waits(nc)
    return nc


_NC_CACHE = {}
_last_exec_ns = None


def _get_nc():
    if "nc" not in _NC_CACHE:
        _NC_CACHE["nc"] = _build_nc()
    return _NC_CACHE["nc"]


# ---------------------------------------------------------------------------
# Optional NTFF profiling hook (used by the local test harness via
# KERNEL_TRACE=1; grading path leaves it off)
# ---------------------------------------------------------------------------
def _install_ntff_hook(so_path="/opt/axon/libaxon_pjrt.so"):
    if "antenv.axon_hooks" in sys.modules:
        return
    try:
        lib = ctypes.CDLL(so_path)
    except OSError:
        lib = None
    if lib is None or not hasattr(lib, "axon_start_nrt_profile"):
        hook = None
    else:
        lib.axon_start_nrt_profile.argtypes = [
            ctypes.POINTER(ctypes.c_int64),
            ctypes.c_size_t,
        ]
        lib.axon_start_nrt_profile.restype = ctypes.c_int64
        lib.axon_stop_nrt_profile.argtypes = [ctypes.c_char_p]
        lib.axon_stop_nrt_profile.restype = ctypes.c_int64

        @contextlib.contextmanager
        def hook(output_dir, device_ids):
            import jax

            jax.devices()
            if device_ids:
                ids = (ctypes.c_int64 * len(device_ids))(*device_ids)
                rc = lib.axon_start_nrt_profile(ids, len(device_ids))
            else:
                rc = lib.axon_start_nrt_profile(None, 0)
            if rc != 0:
                raise RuntimeError(f"axon_start_nrt_profile rc={rc}")
            try:
                yield
            finally:
                n = lib.axon_stop_nrt_profile(str(output_dir).encode())
                print(f"ntff profile: {n} file(s) -> {output_dir}", file=sys.stderr)

    mod = types.ModuleType("antenv.axon_hooks")
    mod.get_axon_ntff_profile_hook = lambda: hook
    sys.modules["antenv.axon_hooks"] = mod


# ---------------------------------------------------------------------------
# Host entry point
# ---------------------------------------------------------------------------
def kernel(hidden_states, position_ids, attention_mask, Wq, Wk, Wv, Wo):
    global _last_exec_ns
    from concourse import bass_utils

    hidden_states = np.asarray(hidden_states, dtype=np.float32)
    position_ids = np.asarray(position_ids)
    attention_mask = np.asarray(attention_mask)
    Wq = np.asarray(Wq, dtype=np.float32)
    Wk = np.asarray(Wk, dtype=np.float32)
    Wv = np.asarray(Wv, dtype=np.float32)
    Wo = np.asarray(Wo, dtype=np.float32)

    if not np.all(np.asarray(attention_mask) > 0):
        # Spec guarantees an all-ones mask; fall back to a host reference
        # implementation for the general case rather than mis-computing.
        return _host_reference(
            hidden_states, position_ids, attention_mask, Wq, Wk, Wv, Wo
        )

    # rope tables per batch: cc = [cos; cos], ssn = [-sin; sin]  (f32 [HD, S])
    half = HD // 2
    inv_freq = 1.0 / (THETA ** (np.arange(0, half, dtype=np.float32) / half))
    ccs, ssns = [], []
    for b in range(B):
        freqs = position_ids[b].astype(np.float32)[:, None] * inv_freq[None, :]
        cosT = np.cos(freqs).T.astype(np.float32)  # [64, S]
        sinT = np.sin(freqs).T.astype(np.float32)
        ccs.append(np.ascontiguousarray(np.concatenate([cosT, cosT], axis=0)))
        ssns.append(np.ascontiguousarray(np.concatenate([-sinT, sinT], axis=0)))

    # multiplicative causal masks for the diagonal blocks: block t in [0, G):
    # dmask[kk, t*QT + qq] = 1 if qq >= t*SC + kk else 0
    kk = np.arange(SC)[:, None]
    qq = np.arange(QT)[None, :]
    dmask = np.concatenate(
        [
            np.where(qq >= t * SC + kk, 1.0, 0.0).astype(np.float16)
            for t in range(G)
        ],
        axis=1,
    )
    dmask = np.ascontiguousarray(dmask)

    # hidden: [p, st, half, c_local, x]  (d = c*128 + p, s = st*QT + x)
    his = []
    for b in range(B):
        hiT = hidden_states[b].T.astype(np.float16)  # [D, S]
        t = hiT.reshape(DMC, 128, NQT, QT).transpose(1, 2, 0, 3)  # [p, st, c, x]
        his.append(np.ascontiguousarray(t.reshape(128, NQT, 2, HALF, QT)))

    in_maps = []
    for c in range(NCORES):
        b = c // KV_SHARDS
        m = c % KV_SHARDS
        FQ = QH * HD
        FKV = KVH * HD
        wq_s = Wq[:, m * FQ : (m + 1) * FQ].astype(np.float16)
        wq_pre = np.ascontiguousarray(
            wq_s.reshape(DMC, 128, QH, HD).transpose(1, 2, 0, 3)
        )  # [p, h, c, x]
        wk_s = Wk[:, m * FKV : (m + 1) * FKV].astype(np.float16)
        wk_pre = np.ascontiguousarray(
            wk_s.reshape(DMC, 128, FKV).transpose(1, 0, 2)
        )  # [p, c, kv*HD]
        wv_s = Wv[:, m * FKV : (m + 1) * FKV].astype(np.float16)
        wv_pre = np.ascontiguousarray(
            wv_s.reshape(DMC, 128, FKV).transpose(1, 0, 2)
        )
        wo_s = Wo[m * FQ : (m + 1) * FQ, :].astype(np.float16)
        wo_pre = np.ascontiguousarray(
            wo_s.reshape(QH, 128, NDT, DT).transpose(1, 2, 0, 3)
        )  # [p, dt, h, x]
        in_maps.append(
            {
                "hi": his[b],
                "wq": wq_pre,
                "wk": wk_pre,
                "wv": wv_pre,
                "wo": wo_pre,
                "cc": ccs[b],
                "ssn": ssns[b],
                "dmask": dmask,
            }
        )

    nc = _get_nc()
    trace = os.environ.get("KERNEL_TRACE", "") == "1"
    if trace:
        _install_ntff_hook()
        bass_utils.upload_artifacts = lambda tmpdir: f"local:{tmpdir}"
    res = bass_utils.run_bass_kernel_spmd(
        nc, in_maps, list(range(NCORES)), trace=trace
    )
    _last_exec_ns = res.exec_time_ns

    out = np.zeros((B, S, D), dtype=np.float32)
    for c in range(NCORES):
        out[c // KV_SHARDS] += np.asarray(res.results[c]["out"], dtype=np.float32)
    return out


def _host_reference(hidden_states, position_ids, attention_mask, Wq, Wk, Wv, Wo):
    """Numpy fallback for inputs outside the spec's guarantees."""
    q = (hidden_states @ Wq).reshape(B, S, H, HD)
    k = (hidden_states @ Wk).reshape(B, S, HKV, HD)
    v = (hidden_states @ Wv).reshape(B, S, HKV, HD)

    half = HD // 2
    inv_freq = 1.0 / (THETA ** (np.arange(0, half, dtype=np.float32) / half))
    freqs = position_ids.astype(np.float32)[..., None] * inv_freq
    cos = np.cos(freqs)[:, :, None, :]
    sin = np.sin(freqs)[:, :, None, :]

    def rope(x):
        x1, x2 = x[..., :half], x[..., half:]
        return np.concatenate([x1 * cos - x2 * sin, x2 * cos + x1 * sin], axis=-1)

    q, k = rope(q), rope(k)
    qg = q.reshape(B, S, HKV, G, HD)
    scores = np.einsum("bqhgd,bkhd->bhgqk", qg, k) * (HD**-0.5)
    causal = np.tril(np.ones((S, S), bool))
    mask = causal[None, None, None] & (attention_mask[:, None, None, None, :] > 0)
    scores = np.where(mask, scores, np.finfo(np.float32).min)
    scores = scores - scores.max(axis=-1, keepdims=True)
    probs = np.exp(scores)
    probs = probs / probs.sum(axis=-1, keepdims=True)
    ctx = np.einsum("bhgqk,bkhd->bqhgd", probs, v).reshape(B, S, H * HD)
    return (ctx @ Wo).astype(np.float32)
